# revision 56
# baseline (speedup 1.0000x reference)
"""Trainium2 Bass kernel for nn_DataEmbedding_v2 (circular conv1d + sinusoidal PE
+ causal threshold-scan "tc" embedding).

Contract: kernel(**inputs) takes FULL inputs (x:(16,2048,64) f32, conv_w:(512,64,3),
tc_w:(512,1), tc_b:(512,)) and returns the FULL (16,2048,512) f32 output.
Internally: batch-sharded data-parallel across 8 NeuronCores (2 batches/core),
params replicated.

v2 structure (vs the 150us baseline): the causal threshold scan's argmax fold
moved off the vector engine onto the PE as bit-pack matmuls: a constant
power-of-two matrix PW packs each 16-token s-chunk's qualifier mask into an
integer C16 = sum(cond * 2^r) accumulated in PSUM per 512-token t-chunk; the
within-chunk argmax is recovered elementwise with an exponent-extraction
(bitcast >> 23) instead of a cross-partition fold.  Everything streams per
512-token chunk (pack -> decode -> tau roundtrip -> conv -> output DMA), with
batch 1's plane interleaved into batch 0's conv so the PE stays dense/warm.

Numerics: the tc threshold compare runs in exact fp32 (bit-matching the
reference's rounding); the bit-pack path is exact (integer C16 <= 65535 in
f32 PSUM, tau <= 2048 exact in fp16); conv weights/activations and pe run in
fp16 on the PE; tau*tc_w uses an fp16 hi+lo split of tc_w.
"""

import math
import os
import sys

sys.path.insert(0, "/opt/trn_rl_repo")

import numpy as np

B, S, C, D = 16, 2048, 64, 512
NCORES = 8
BLOC = B // NCORES  # batches per core
P = 128
NT = S // P  # 16 tiles of 128 tokens
NCH = 4  # t-chunks
CW = S // NCH  # 512 tokens per chunk
ETA = 0.3
EPS = 1e-08
# gpsimd tensor ops measured ~12.6ns/col on HW (6-25x slower than DVE) and
# degrade concurrent DVE throughput via SBUF port contention -- keep the
# plane entirely on the vector engine.
GP_J = ()


def _emit(tc, aps):
    """Emit the per-core Tile kernel. aps: dict of DRAM APs."""
    from contextlib import ExitStack

    from concourse import mybir

    f32 = mybir.dt.float32
    f16 = mybir.dt.float16
    u32 = mybir.dt.uint32
    Alu = mybir.AluOpType
    Ax = mybir.AxisListType

    nc = tc.nc
    xin, pe16, wt01, wt2 = aps["xin"], aps["pe16"], aps["wt01"], aps["wt2"]
    identh, causal = aps["identh"], aps["causal"]
    t1c, pw, qb, out = aps["t1c"], aps["pw"], aps["qb"], aps["out"]

    with ExitStack() as ctx:
        singles = ctx.enter_context(tc.tile_pool(name="singles", bufs=1))
        xpool = ctx.enter_context(tc.tile_pool(name="xpool", bufs=2))
        small = ctx.enter_context(tc.tile_pool(name="small", bufs=2))
        rowpool = ctx.enter_context(tc.tile_pool(name="rowpool", bufs=2))
        xtp = ctx.enter_context(tc.tile_pool(name="xtp", bufs=2))
        gjp = ctx.enter_context(tc.tile_pool(name="gjp", bufs=3))
        decp = ctx.enter_context(tc.tile_pool(name="decp", bufs=2))
        outp = ctx.enter_context(tc.tile_pool(name="outp", bufs=2))
        psW = ctx.enter_context(tc.tile_pool(name="psW", bufs=3, space="PSUM"))
        psPK = ctx.enter_context(tc.tile_pool(name="psPK", bufs=4, space="PSUM"))
        psT = ctx.enter_context(tc.tile_pool(name="psT", bufs=1, space="PSUM"))

        # ---- x loads first (sync ring), then small consts (sync), pe16 on
        # the scalar ring so the big PE table doesn't block the critical path.
        xins = {}
        for b in range(BLOC):
            xin_sb = xpool.tile([P, NT, C], f32, tag="xin", name=f"xin_sb{b}")
            nc.sync.dma_start(xin_sb, xin[b].rearrange("(j p) c -> p j c", p=P))
            xins[b] = xin_sb
        identh_sb = singles.tile([P, P], f16)
        nc.sync.dma_start(identh_sb, identh)
        causal_sb = singles.tile([P, P], f16)
        nc.sync.dma_start(causal_sb, causal)
        selw_sb = singles.tile([P, S], f16)
        nc.sync.dma_start(selw_sb, aps["selw"])
        t1_sb = singles.tile([P, NT], f32)
        nc.sync.dma_start(t1_sb, t1c)
        qb_sb = singles.tile([P, 1], f32)
        nc.sync.dma_start(qb_sb, qb)
        wt01_sb = singles.tile([P, D], f16)
        nc.sync.dma_start(wt01_sb, wt01)
        wt2_sb = singles.tile([P, D], f16)
        nc.sync.dma_start(wt2_sb[C : 2 * C, :], wt2)
        tcw8_sb = singles.tile([8, NCH, D], f16)
        nc.sync.dma_start(tcw8_sb, aps["tcw8"])
        pw_sb = singles.tile([P, S], f16)
        nc.sync.dma_start(pw_sb, pw)

        # ---- PE priming: absorb each const-DMA wait on a dedicated PE op so
        # later matmuls never carry more than one sync wait.  Only the consts
        # needed by the throw broadcast prime here; the plane/conv weights
        # prime later so they don't stall the startup-critical PE chain.
        prime_h = psT.tile([P, P], f16, tag="pst")
        nc.tensor.transpose(prime_h, identh_sb, identh_sb)
        dumps = singles.tile([P, 4], f32)
        nc.scalar.copy(dumps[:, 0:1], prime_h[:, 0:1])
        nc.tensor.ldweights(selw_sb[:, 0:P])
        nc.tensor.ldweights(wt01_sb[:, 0:P])
        nc.tensor.ldweights(wt2_sb[C : 2 * C, 0:P], tile_position=(64, 0))
        nc.tensor.ldweights(tcw8_sb[:, 0, 0:P])
        nc.tensor.ldweights(pw_sb[:, 0:P])

        # ---- Phase A (per batch): norms/thresh/broadcast + xTA build ----
        st = {}

        def phase_a(b):
            xin_sb = xins[b]
            # norms (L1 over channels), two-level sum (bit-matches reference)
            r8 = small.tile([P, NT, 8], f32, tag="r8", name=f"r8_{b}")
            nc.vector.tensor_reduce(
                r8,
                xin_sb.rearrange("p j (a b) -> p j a b", b=8),
                axis=Ax.X,
                op=Alu.add,
                apply_absolute_value=True,
            )
            normc = small.tile([P, NT], f32, tag="normc", name=f"normc{b}")
            nc.vector.tensor_reduce(normc, r8, axis=Ax.X, op=Alu.add)
            # thresh = (norms + EPS) * 0.7 (exact fp32 rounding order of ref)
            thc = small.tile([P, NT], f32, tag="thc", name=f"thc{b}")
            nc.vector.tensor_scalar(
                thc, normc, float(EPS), float(1.0 - ETA), op0=Alu.add, op1=Alu.mult
            )
            # thresh broadcast over partitions: short DRAM roundtrip for the
            # s-contiguous row, then a K=1 ones-matmul fans it out on the PE
            # (the 1MB partition_broadcast DMA costs ~30us; this costs ~5us)
            # thresh broadcast over partitions: exact 3-way f16 split
            # (hi + mid/4096 + lo/4096 == thc bit-exact; the scaling keeps
            # the f16 components normal), short DRAM roundtrip for the
            # s-contiguous rows, then a K=3 ones-matmul fans out on the PE.
            # (partition_broadcast DMA of the 1MB plane costs ~30us serial.)
            th3 = small.tile([P, NT, 8], f16, tag="th3", name=f"th3_{b}")
            nc.vector.memset(th3, 0)
            r1 = small.tile([P, NT], f32, tag="r1", name=f"r1_{b}")
            r2n = small.tile([P, NT], f32, tag="r2n", name=f"r2n_{b}")
            nc.vector.tensor_copy(th3[:, :, 0], thc)
            nc.vector.tensor_tensor(r1, thc, th3[:, :, 0], op=Alu.subtract)
            nc.vector.tensor_scalar(th3[:, :, 1], r1, 4096.0, None, op0=Alu.mult)
            nc.vector.scalar_tensor_tensor(
                r2n, th3[:, :, 1], float(2.0**-12), r1,
                op0=Alu.mult, op1=Alu.subtract,
            )
            nc.vector.tensor_scalar(th3[:, :, 2], r2n, -4096.0, None, op0=Alu.mult)
            # on-chip permutation: transpose [P, (j k)] -> [(j k), P] puts
            # tile j's three components at partitions 8j..8j+2 (k=3..7 zero);
            # the selw matmul (3 nonzero rows per output tile) fans them out
            th8T_ps = psT.tile([P, P], f16, tag="pst", name=f"th8Tps{b}")
            nc.tensor.transpose(
                th8T_ps, th3.rearrange("p j k -> p (j k)"), identh_sb
            )
            th8T = rowpool.tile([P, P], f16, tag="throwrow", name=f"th8T{b}")
            nc.scalar.copy(th8T, th8T_ps)
            throw = rowpool.tile([P, S], f32, tag="throw", name=f"throw{b}")
            for g in range(NCH):
                thps = psW.tile([P, CW], f32, tag="work", name=f"thps{b}_{g}")
                for m in range(4):
                    j = 4 * g + m
                    nc.tensor.matmul(
                        thps[:, m * P : (m + 1) * P],
                        lhsT=selw_sb[:, j * P : (j + 1) * P],
                        rhs=th8T,
                        start=(m == 0),
                        stop=(m == 3),
                    )
                nc.scalar.copy(throw[:, g * CW : (g + 1) * CW], thps)

            # xTA: fp16 x, channel-major, two stacked tap views.
            # 4 transposes share one PSUM bank, evacuated by 2 wide ACT copies.
            xin16 = xpool.tile([P, NT, C], f16, tag="xin16", name=f"xin16_{b}")
            nc.vector.tensor_copy(xin16, xin_sb)
            nc.tensor.ldweights(xin16[:, 0, :])  # absorb DVE wait for PE
            xTA = xtp.tile([P, S + 1], f16, tag="xta", name=f"xTA{b}")
            for g in range(4):
                xtg = psW.tile([C, 4, P], f16, tag="work", name=f"xtg{b}_{g}")
                for m in range(4):
                    nc.tensor.transpose(
                        xtg[:, m, :], xin16[:, 4 * g + m, :], identh_sb
                    )
                c0 = g * 4 * P
                nc.scalar.copy(
                    xTA[0:C, 1 + c0 : 1 + c0 + 4 * P],
                    xtg.rearrange("p m q -> p (m q)"),
                )
                nc.scalar.copy(
                    xTA[C : 2 * C, c0 : c0 + 4 * P],
                    xtg.rearrange("p m q -> p (m q)"),
                )
            nc.scalar.copy(xTA[0:C, 0:1], xTA[0:C, S : S + 1])
            nc.scalar.copy(xTA[C : 2 * C, S : S + 1], xTA[C : 2 * C, 0:1])
            st[b] = (normc, throw, xTA)

        pe16_sb = singles.tile([P, NT, D], f16)

        # ---- streamed plane + pack + decode + tau + conv ----
        def plane_steps(b, jlist):
            """Emit plane tiles (compare + causal + bit-pack matmuls) for the
            given j's of batch b, plus deferred chunk-close decode work."""
            normc, throw, _ = st[b]
            pks, rcs, taucs = state[b]["pks"], state[b]["rc"], state[b]["tauc"]
            for j in jlist:
                c0 = j * P
                w = S - c0
                gj = gjp.tile([P, S], f16, tag="gj", name=f"gj{b}_{j}")
                # cond[s, t] = thresh[t] > norms[s]  (exact fp32 compare)
                nc.vector.tensor_scalar(
                    gj[:, 0:w], throw[:, c0:S], normc[:, j : j + 1], None,
                    op0=Alu.is_gt,
                )
                # causal mask inside the diagonal 128 columns
                nc.vector.tensor_tensor(
                    gj[:, 0:P], gj[:, 0:P], causal_sb, op=Alu.mult
                )
                # bit-pack matmuls: accumulate 16-wide qualifier masks into
                # every open t-chunk this plane tile touches
                for tcn in range(j // 4, NCH):
                    tlo, thi = max(c0, tcn * CW), (tcn + 1) * CW
                    if j == 0:
                        pk = psPK.tile([P, CW], f32, tag="pk",
                                       name=f"pk{b}_{tcn}")
                        pks[tcn] = pk
                    nc.tensor.matmul(
                        pks[tcn][:, tlo - tcn * CW : thi - tcn * CW],
                        lhsT=pw_sb[:, c0 : c0 + P],
                        rhs=gj[:, tlo - c0 : thi - c0],
                        start=(j == 0),
                        stop=(j == 4 * tcn + 3),
                    )
                # deferred chunk-close: decode chunk (j-7)//4 one chunk late
                # so the DVE never stalls on the ACT evacuation
                if (j + 1) % 4 == 0 and j >= 7:
                    chunk_close(b, (j - 7) // 4)

        def chunk_close(b, tcn):
            """Evacuate + decode chunk tcn of batch b: C16 -> F -> rc -> tau."""
            pks, rc, tauc = state[b]["pks"], state[b]["rc"], state[b]["tauc"]
            c1 = decp.tile([P, CW], f32, tag="c1", name=f"c1_{b}_{tcn}")
            nc.scalar.copy(c1, pks[tcn])
            # mask = [C16 > 0] via ACT Sign (exact: C16 is a non-negative
            # integer), freeing the vector engine mid-pipeline
            mask = decp.tile([P, CW], f16, tag="mask", name=f"mask{b}_{tcn}")
            nc.scalar.sign(mask, c1)
            eu = decp.tile([P, CW], u32, tag="eu", name=f"eu{b}_{tcn}")
            nc.vector.tensor_scalar(
                eu, c1.bitcast(u32), 23, None, op0=Alu.logical_shift_right
            )
            # F = (exponent + 16q - 126) * mask = (s_max within chunk q) + 1
            # (the uint32 exponent feeds the arithmetic op directly -- only
            # bitVec ops forbid the input dtype cast)
            ff = decp.tile([P, CW], f16, tag="ff", name=f"ff{b}_{tcn}")
            nc.vector.scalar_tensor_tensor(
                ff, eu, qb_sb, mask, op0=Alu.add, op1=Alu.mult
            )
            # cross-partition max via PE transpose + free-dim reduce
            pT = psT.tile([P, 4, P], f16, tag="pst", name=f"pT{b}_{tcn}")
            for m in range(4):
                nc.tensor.transpose(
                    pT[:, m, :], ff[:, m * P : (m + 1) * P], identh_sb
                )
            nc.vector.tensor_reduce(
                rc[:, 4 * tcn : 4 * tcn + 4], pT, axis=Ax.X, op=Alu.max
            )
            # tau = (rc > 0) ? (t + 1 - rc) : 0
            sl = slice(4 * tcn, 4 * tcn + 4)
            m01 = state[b]["m01"]
            td = state[b]["td"]
            nc.vector.tensor_scalar(m01[:, sl], rc[:, sl], 0.0, None, op0=Alu.is_gt)
            nc.vector.tensor_tensor(td[:, sl], t1_sb[:, sl], rc[:, sl],
                                    op=Alu.subtract)
            nc.vector.tensor_tensor(tauc[:, sl], td[:, sl], m01[:, sl],
                                    op=Alu.mult)
            # on-chip tau transport: duplicate each of the 4 tile-columns
            # (for the hi/lo weight rows), transpose to [8, P] at base 0 --
            # tau <= 2047 is exact in fp16
            tauc8 = decp.tile([P, 4, 2], f16, tag="tauc8",
                              name=f"tauc8_{b}_{tcn}")
            nc.vector.tensor_copy(tauc8[:, :, 0], tauc[:, sl])
            nc.vector.tensor_copy(tauc8[:, :, 1], tauc[:, sl])
            tauT_ps = psT.tile([8, P], f16, tag="pst", name=f"tauTps{b}_{tcn}")
            nc.tensor.transpose(
                tauT_ps, tauc8.rearrange("p a k -> p (a k)"), identh_sb
            )
            tauT8 = decp.tile([8, P], f16, tag="tauT8", bufs=4,
                              name=f"tauT8_{b}_{tcn}")
            nc.scalar.copy(tauT8, tauT_ps)
            state[b]["tauT8"][tcn] = tauT8

        def conv_chunk(b, tcn):
            """Conv + pe + tau matmuls, evacuation and output DMA for the 4
            token tiles of chunk tcn of batch b."""
            _, _, xTA = st[b]
            tauT8 = state[b]["tauT8"][tcn]
            if b == 0 and tcn == 0:
                # absorb the pe16 const-DMA wait before the first conv matmul
                nc.tensor.ldweights(pe16_sb[:, 0, 0:P])
            osb4 = outp.tile([P, 4, D], f32, tag="osb", name=f"osb{b}_{tcn}")
            for jj in range(4):
                i = 4 * tcn + jj
                ps = psW.tile([P, D], f32, tag="work", name=f"ps{b}_{i}")
                nc.tensor.matmul(
                    ps, lhsT=identh_sb, rhs=pe16_sb[:, i, :], start=True,
                    stop=False,
                )
                nc.tensor.matmul(
                    ps,
                    lhsT=xTA[:, i * P : (i + 1) * P],
                    rhs=wt01_sb,
                    start=False,
                    stop=False,
                )
                nc.tensor.matmul(
                    ps,
                    lhsT=xTA[C : 2 * C, i * P + 1 : (i + 1) * P + 1],
                    rhs=wt2_sb[C : 2 * C, :],
                    start=False,
                    stop=False,
                )
                nc.tensor.matmul(
                    ps,
                    lhsT=tauT8,
                    rhs=tcw8_sb[:, jj, :],
                    start=False,
                    stop=True,
                    tile_position=(0, 0),
                )
                nc.scalar.copy(osb4[:, jj, :], ps)
                if jj == 1:
                    # first half ships as soon as its two tiles are staged;
                    # two HWDGE rings so the packets interleave across SDMA
                    ov = out[b, tcn * CW : (tcn + 1) * CW, :].rearrange(
                        "(jj p) d -> p jj d", p=P
                    )
                    nc.sync.dma_start(ov[:, 0:2, :], osb4[:, 0:2, :])
            nc.scalar.dma_start(ov[:, 2:4, :], osb4[:, 2:4, :])

        state = {}
        for b in range(BLOC):
            rc = small.tile([P, NT], f32, tag="rc", name=f"rc{b}")
            m01 = small.tile([P, NT], f32, tag="m01", name=f"m01_{b}")
            td = small.tile([P, NT], f32, tag="td", name=f"td{b}")
            tauc = small.tile([P, NT], f32, tag="tauc", name=f"tauc{b}")
            state[b] = dict(pks={}, rc=rc, m01=m01, td=td, tauc=tauc,
                            tauT8={})

        # conv chunks slot into the plane j-loop as soon as their tau
        # transport can have landed, keeping the PE dense (HAM stays warm);
        # batch 0's plane starts before batch 1's phase A, and the pe16 load
        # dispatches after the startup-critical sync-ring traffic
        phase_a(0)
        phase_a(1)
        nc.scalar.dma_start(pe16_sb, pe16.rearrange("(i p) d -> p i d", p=P))
        plane_steps(0, range(0, 12))
        conv_chunk(0, 0)
        plane_steps(0, range(12, 16))
        conv_chunk(0, 1)
        chunk_close(0, 3)
        plane_steps(1, range(0, 4))
        conv_chunk(0, 2)
        plane_steps(1, range(4, 8))
        conv_chunk(0, 3)
        plane_steps(1, range(8, 12))
        conv_chunk(1, 0)
        plane_steps(1, range(12, 16))
        conv_chunk(1, 1)
        chunk_close(1, 3)
        conv_chunk(1, 2)
        conv_chunk(1, 3)


def build_bass():
    """Build the per-core Bass module (traced once, then bacc-compiled)."""
    import concourse.tile as tile
    from concourse import bacc, mybir

    f32 = mybir.dt.float32
    f16 = mybir.dt.float16

    nc = bacc.Bacc(
        "TRN2",
        target_bir_lowering=False,
        debug=False,
        enable_asserts=False,
        num_devices=NCORES,
    )
    aps = {}
    aps["xin"] = nc.dram_tensor("xin", (BLOC, S, C), f32, kind="ExternalInput").ap()
    aps["pe16"] = nc.dram_tensor("pe16", (S, D), f16, kind="ExternalInput").ap()
    aps["wt01"] = nc.dram_tensor("wt01", (P, D), f16, kind="ExternalInput").ap()
    aps["wt2"] = nc.dram_tensor("wt2", (C, D), f16, kind="ExternalInput").ap()
    aps["tcw8"] = nc.dram_tensor("tcw8", (8, NCH, D), f16, kind="ExternalInput").ap()
    aps["identh"] = nc.dram_tensor("identh", (P, P), f16, kind="ExternalInput").ap()
    aps["causal"] = nc.dram_tensor("causal", (P, P), f16, kind="ExternalInput").ap()
    aps["t1c"] = nc.dram_tensor("t1c", (P, NT), f32, kind="ExternalInput").ap()
    aps["pw"] = nc.dram_tensor("pw", (P, S), f16, kind="ExternalInput").ap()
    aps["selw"] = nc.dram_tensor("selw", (P, S), f16, kind="ExternalInput").ap()
    aps["qb"] = nc.dram_tensor("qb", (P, 1), f32, kind="ExternalInput").ap()
    aps["out"] = nc.dram_tensor("out", (BLOC, S, D), f32, kind="ExternalOutput").ap()

    with tile.TileContext(nc) as tc:
        _emit(tc, aps)
    nc.compile()
    return nc


def make_consts():
    """Host-side constant tensors (replicated params + index helpers)."""
    # positional embedding, matching the reference formula in fp32
    position = np.arange(S, dtype=np.float32)[:, None]
    div_term = np.exp(
        np.arange(0, D, 2, dtype=np.float32) * np.float32(-math.log(10000.0) / D)
    ).astype(np.float32)
    ang = (position * div_term).astype(np.float32)
    pe = np.zeros((S, D), dtype=np.float32)
    pe[:, 0::2] = np.sin(ang)
    pe[:, 1::2] = np.cos(ang)

    sl = np.arange(P, dtype=np.float32)[:, None]
    jj = np.arange(NT, dtype=np.float32)[None, :]
    # bit-pack matrix: PW[p, 128j + m] = 2^(p%16) if m == 8j + p//16
    pwm = np.zeros((P, S), dtype=np.float16)
    for j in range(NT):
        for p in range(P):
            pwm[p, 128 * j + 8 * j + p // 16] = np.float16(2.0 ** (p % 16))
    # throw-broadcast selector: column block j selects the 3 split components
    # of tile j (rows 8j..8j+2) with weights (1, 2^-12, 2^-12)
    selw = np.zeros((P, S), dtype=np.float16)
    for j in range(NT):
        for k, w in enumerate((1.0, 2.0**-12, 2.0**-12)):
            selw[8 * j + k, 128 * j : 128 * (j + 1)] = np.float16(w)
    consts = {
        "identh": np.eye(P, dtype=np.float16),
        "causal": (np.arange(P)[:, None] < np.arange(P)[None, :]).astype(np.float16),
        "t1c": (jj * P + sl + 1.0).astype(np.float32),
        "pw": pwm,
        "selw": selw,
        "qb": (16.0 * np.arange(P, dtype=np.float32) - 126.0)[:, None],
    }
    return pe, consts


def make_shared_inputs(conv_w, tc_w, tc_b):
    pe, consts = make_consts()
    pe_b = (pe + np.asarray(tc_b, np.float32)[None, :]).astype(np.float32)
    # conv weights, channel-major per tap: wk[c, d] = conv_w[d, c, k]
    wt = np.transpose(np.asarray(conv_w, np.float32), (2, 1, 0))  # (k, c, d)
    wt01 = np.concatenate([wt[0], wt[1]], axis=0).astype(np.float16)  # (128, D)
    wt2 = wt[2].astype(np.float16)  # (64, D)
    # tc_w split into fp16 hi+lo (tau amplifies errors by up to 2048);
    # tcw8[2jj, jj] = w_hi, tcw8[2jj+1, jj] = w_lo selects tile-in-chunk jj
    w = np.asarray(tc_w, np.float32)[:, 0]
    w_hi = w.astype(np.float16)
    w_lo = (w - w_hi.astype(np.float32)).astype(np.float16)
    tcw8 = np.zeros((8, NCH, D), dtype=np.float16)
    for jj in range(NCH):
        tcw8[2 * jj, jj, :] = w_hi
        tcw8[2 * jj + 1, jj, :] = w_lo
    return {
        "pe16": pe_b.astype(np.float16),
        "wt01": np.ascontiguousarray(wt01),
        "wt2": np.ascontiguousarray(wt2),
        "tcw8": tcw8,
        **{k: np.ascontiguousarray(v) for k, v in consts.items()},
    }


_BUILD_CACHE = {}


def _install_ntff_hook():
    """The agent image's antenv lacks axon_hooks; synthesize it from the
    boot module's ctypes implementation so trace=True works under axon."""
    import sys as _sys
    import types

    if "antenv.axon_hooks" in _sys.modules:
        return
    try:
        from trn_agent_boot.trn_boot import _ntff_profile_via_ctypes

        hook = _ntff_profile_via_ctypes("/opt/axon/libaxon_pjrt.so")
        m = types.ModuleType("antenv.axon_hooks")
        m.get_axon_ntff_profile_hook = lambda: hook
        _sys.modules["antenv.axon_hooks"] = m
    except Exception as e:  # degrade to no-trace
        print("[kernel] ntff hook install failed:", e)


def kernel(x, conv_w, tc_w, tc_b):
    x = np.ascontiguousarray(np.asarray(x, dtype=np.float32))
    conv_w = np.asarray(conv_w, dtype=np.float32)
    tc_w = np.asarray(tc_w, dtype=np.float32)
    tc_b = np.asarray(tc_b, dtype=np.float32)
    assert x.shape == (B, S, C), x.shape

    from concourse.bass_utils import run_bass_kernel_spmd

    if "nc" not in _BUILD_CACHE:
        _BUILD_CACHE["nc"] = build_bass()
    nc = _BUILD_CACHE["nc"]

    shared = make_shared_inputs(conv_w, tc_w, tc_b)
    in_maps = []
    for c in range(NCORES):
        m = dict(shared)
        m["xin"] = np.ascontiguousarray(x[c * BLOC : (c + 1) * BLOC])
        in_maps.append(m)

    trace = bool(int(os.environ.get("KERNEL_TRACE", "0")))
    if trace:
        _install_ntff_hook()
    res = run_bass_kernel_spmd(
        nc, in_maps, core_ids=list(range(NCORES)), trace=trace, trace_cores=[0]
    )
    if trace and res.exec_time_ns is not None:
        print(
            f"[kernel] HW exec time: {res.exec_time_ns} ns "
            f"(mean {res.mean_exec_time_ns} ns)"
        )
        kernel.last_exec_time_ns = res.exec_time_ns
        kernel.last_trace = res.instructions_and_trace
    out = np.concatenate([r["out"] for r in res.results], axis=0)
    return out


if __name__ == "__main__":
    build_bass()
    print("build ok")


# revision 57
# speedup vs baseline: 1.0842x; 1.0842x over previous
"""Trainium2 Bass kernel for nn_DataEmbedding_v2 (circular conv1d + sinusoidal PE
+ causal threshold-scan "tc" embedding).

Contract: kernel(**inputs) takes FULL inputs (x:(16,2048,64) f32, conv_w:(512,64,3),
tc_w:(512,1), tc_b:(512,)) and returns the FULL (16,2048,512) f32 output.
Internally: batch-sharded data-parallel across 8 NeuronCores (2 batches/core),
params replicated.

v2 structure (vs the 150us baseline): the causal threshold scan's argmax fold
moved off the vector engine onto the PE as bit-pack matmuls: a constant
power-of-two matrix PW packs each 16-token s-chunk's qualifier mask into an
integer C16 = sum(cond * 2^r) accumulated in PSUM per 512-token t-chunk; the
within-chunk argmax is recovered elementwise with an exponent-extraction
(bitcast >> 23) instead of a cross-partition fold.  Everything streams per
512-token chunk (pack -> decode -> tau roundtrip -> conv -> output DMA), with
batch 1's plane interleaved into batch 0's conv so the PE stays dense/warm.

Numerics: the tc threshold compare runs in exact fp32 (bit-matching the
reference's rounding); the bit-pack path is exact (integer C16 <= 65535 in
f32 PSUM, tau <= 2048 exact in fp16); conv weights/activations and pe run in
fp16 on the PE; tau*tc_w uses an fp16 hi+lo split of tc_w.
"""

import math
import os
import sys

sys.path.insert(0, "/opt/trn_rl_repo")

import numpy as np

B, S, C, D = 16, 2048, 64, 512
NCORES = 8
BLOC = B // NCORES  # batches per core
P = 128
NT = S // P  # 16 tiles of 128 tokens
NCH = 4  # t-chunks
CW = S // NCH  # 512 tokens per chunk
ETA = 0.3
EPS = 1e-08
# gpsimd tensor ops measured ~12.6ns/col on HW (6-25x slower than DVE) and
# degrade concurrent DVE throughput via SBUF port contention -- keep the
# plane entirely on the vector engine.
GP_J = ()


def _emit(tc, aps):
    """Emit the per-core Tile kernel. aps: dict of DRAM APs."""
    from contextlib import ExitStack

    from concourse import mybir

    f32 = mybir.dt.float32
    f16 = mybir.dt.float16
    u32 = mybir.dt.uint32
    Alu = mybir.AluOpType
    Ax = mybir.AxisListType

    nc = tc.nc
    xin, pe16, wt01, wt2 = aps["xin"], aps["pe16"], aps["wt01"], aps["wt2"]
    identh, causal = aps["identh"], aps["causal"]
    t1c, pw, qb, out = aps["t1c"], aps["pw"], aps["qb"], aps["out"]

    with ExitStack() as ctx:
        singles = ctx.enter_context(tc.tile_pool(name="singles", bufs=1))
        xpool = ctx.enter_context(tc.tile_pool(name="xpool", bufs=2))
        small = ctx.enter_context(tc.tile_pool(name="small", bufs=2))
        rowpool = ctx.enter_context(tc.tile_pool(name="rowpool", bufs=2))
        xtp = ctx.enter_context(tc.tile_pool(name="xtp", bufs=2))
        gjp = ctx.enter_context(tc.tile_pool(name="gjp", bufs=3))
        decp = ctx.enter_context(tc.tile_pool(name="decp", bufs=2))
        outp = ctx.enter_context(tc.tile_pool(name="outp", bufs=2))
        psW = ctx.enter_context(tc.tile_pool(name="psW", bufs=3, space="PSUM"))
        psPK = ctx.enter_context(tc.tile_pool(name="psPK", bufs=4, space="PSUM"))
        psT = ctx.enter_context(tc.tile_pool(name="psT", bufs=1, space="PSUM"))

        # ---- x loads first (sync ring), then small consts (sync), pe16 on
        # the scalar ring so the big PE table doesn't block the critical path.
        xins = {}
        for b in range(BLOC):
            xin_sb = xpool.tile([P, NT, C], f32, tag="xin", name=f"xin_sb{b}")
            nc.sync.dma_start(xin_sb, xin[b].rearrange("(j p) c -> p j c", p=P))
            xins[b] = xin_sb
        identh_sb = singles.tile([P, P], f16)
        nc.sync.dma_start(identh_sb, identh)
        causal_sb = singles.tile([P, P], f16)
        nc.sync.dma_start(causal_sb, causal)
        selw_sb = singles.tile([P, S], f16)
        nc.sync.dma_start(selw_sb, aps["selw"])
        t1_sb = singles.tile([P, NT], f32)
        nc.sync.dma_start(t1_sb, t1c)
        qb_sb = singles.tile([P, 1], f32)
        nc.sync.dma_start(qb_sb, qb)
        wt01_sb = singles.tile([P, D], f16)
        nc.sync.dma_start(wt01_sb, wt01)
        wt2_sb = singles.tile([P, D], f16)
        nc.sync.dma_start(wt2_sb[C : 2 * C, :], wt2)
        tcw8_sb = singles.tile([8, NCH, D], f16)
        nc.sync.dma_start(tcw8_sb, aps["tcw8"])
        pw_sb = singles.tile([P, S], f16)
        nc.sync.dma_start(pw_sb, pw)

        # ---- PE priming: absorb each const-DMA wait on a dedicated PE op so
        # later matmuls never carry more than one sync wait.  Only the consts
        # needed by the throw broadcast prime here; the plane/conv weights
        # prime later so they don't stall the startup-critical PE chain.
        prime_h = psT.tile([P, P], f16, tag="pst")
        nc.tensor.transpose(prime_h, identh_sb, identh_sb)
        dumps = singles.tile([P, 4], f32)
        nc.scalar.copy(dumps[:, 0:1], prime_h[:, 0:1])
        nc.tensor.ldweights(selw_sb[:, 0:P])
        nc.tensor.ldweights(wt01_sb[:, 0:P])
        nc.tensor.ldweights(wt2_sb[C : 2 * C, 0:P], tile_position=(64, 0))
        nc.tensor.ldweights(tcw8_sb[:, 0, 0:P])
        nc.tensor.ldweights(pw_sb[:, 0:P])

        # ---- Phase A (per batch): norms/thresh/broadcast + xTA build ----
        st = {}

        def phase_a(b):
            xin_sb = xins[b]
            # norms (L1 over channels), two-level sum (bit-matches reference)
            r8 = small.tile([P, NT, 8], f32, tag="r8", name=f"r8_{b}")
            nc.vector.tensor_reduce(
                r8,
                xin_sb.rearrange("p j (a b) -> p j a b", b=8),
                axis=Ax.X,
                op=Alu.add,
                apply_absolute_value=True,
            )
            normc = small.tile([P, NT], f32, tag="normc", name=f"normc{b}")
            nc.vector.tensor_reduce(normc, r8, axis=Ax.X, op=Alu.add)
            # thresh = (norms + EPS) * 0.7 (exact fp32 rounding order of ref)
            thc = small.tile([P, NT], f32, tag="thc", name=f"thc{b}")
            nc.vector.tensor_scalar(
                thc, normc, float(EPS), float(1.0 - ETA), op0=Alu.add, op1=Alu.mult
            )
            # thresh broadcast over partitions: short DRAM roundtrip for the
            # s-contiguous row, then a K=1 ones-matmul fans it out on the PE
            # (the 1MB partition_broadcast DMA costs ~30us; this costs ~5us)
            # thresh broadcast over partitions: exact 3-way f16 split
            # (hi + mid/4096 + lo/4096 == thc bit-exact; the scaling keeps
            # the f16 components normal), short DRAM roundtrip for the
            # s-contiguous rows, then a K=3 ones-matmul fans out on the PE.
            # (partition_broadcast DMA of the 1MB plane costs ~30us serial.)
            th3 = small.tile([P, NT, 8], f16, tag="th3", name=f"th3_{b}")
            nc.vector.memset(th3, 0)
            r1 = small.tile([P, NT], f32, tag="r1", name=f"r1_{b}")
            r2n = small.tile([P, NT], f32, tag="r2n", name=f"r2n_{b}")
            nc.vector.tensor_copy(th3[:, :, 0], thc)
            nc.vector.tensor_tensor(r1, thc, th3[:, :, 0], op=Alu.subtract)
            nc.vector.tensor_scalar(th3[:, :, 1], r1, 4096.0, None, op0=Alu.mult)
            nc.vector.scalar_tensor_tensor(
                r2n, th3[:, :, 1], float(2.0**-12), r1,
                op0=Alu.mult, op1=Alu.subtract,
            )
            nc.vector.tensor_scalar(th3[:, :, 2], r2n, -4096.0, None, op0=Alu.mult)
            # on-chip permutation: transpose [P, (j k)] -> [(j k), P] puts
            # tile j's three components at partitions 8j..8j+2 (k=3..7 zero);
            # the selw matmul (3 nonzero rows per output tile) fans them out
            th8T_ps = psT.tile([P, P], f16, tag="pst", name=f"th8Tps{b}")
            nc.tensor.transpose(
                th8T_ps, th3.rearrange("p j k -> p (j k)"), identh_sb
            )
            th8T = rowpool.tile([P, P], f16, tag="throwrow", name=f"th8T{b}")
            nc.scalar.copy(th8T, th8T_ps)
            throw = rowpool.tile([P, S], f32, tag="throw", name=f"throw{b}")
            for g in range(NCH):
                thps = psW.tile([P, CW], f32, tag="work", name=f"thps{b}_{g}")
                for m in range(4):
                    j = 4 * g + m
                    nc.tensor.matmul(
                        thps[:, m * P : (m + 1) * P],
                        lhsT=selw_sb[:, j * P : (j + 1) * P],
                        rhs=th8T,
                        start=(m == 0),
                        stop=(m == 3),
                    )
                nc.scalar.copy(throw[:, g * CW : (g + 1) * CW], thps)

            # xTA: fp16 x, channel-major, two stacked tap views.
            # 4 transposes share one PSUM bank, evacuated by 2 wide ACT copies.
            xin16 = xpool.tile([P, NT, C], f16, tag="xin16", name=f"xin16_{b}")
            nc.vector.tensor_copy(xin16, xin_sb)
            nc.tensor.ldweights(xin16[:, 0, :])  # absorb DVE wait for PE
            xTA = xtp.tile([P, S + 1], f16, tag="xta", name=f"xTA{b}")
            for g in range(4):
                xtg = psW.tile([C, 4, P], f16, tag="work", name=f"xtg{b}_{g}")
                for m in range(4):
                    nc.tensor.transpose(
                        xtg[:, m, :], xin16[:, 4 * g + m, :], identh_sb
                    )
                c0 = g * 4 * P
                nc.scalar.copy(
                    xTA[0:C, 1 + c0 : 1 + c0 + 4 * P],
                    xtg.rearrange("p m q -> p (m q)"),
                )
                nc.scalar.copy(
                    xTA[C : 2 * C, c0 : c0 + 4 * P],
                    xtg.rearrange("p m q -> p (m q)"),
                )
            nc.scalar.copy(xTA[0:C, 0:1], xTA[0:C, S : S + 1])
            nc.scalar.copy(xTA[C : 2 * C, S : S + 1], xTA[C : 2 * C, 0:1])
            st[b] = (normc, throw, xTA)

        pe16_sb = singles.tile([P, NT, D], f16)

        # ---- streamed plane + pack + decode + tau + conv ----
        def plane_steps(b, jlist):
            """Emit plane tiles (compare + causal + bit-pack matmuls) for the
            given j's of batch b, plus deferred chunk-close decode work."""
            normc, throw, _ = st[b]
            pks, rcs, taucs = state[b]["pks"], state[b]["rc"], state[b]["tauc"]
            for j in jlist:
                c0 = j * P
                w = S - c0
                gj = gjp.tile([P, S], f16, tag="gj", name=f"gj{b}_{j}")
                # cond[s, t] = thresh[t] > norms[s]  (exact fp32 compare)
                nc.vector.tensor_scalar(
                    gj[:, 0:w], throw[:, c0:S], normc[:, j : j + 1], None,
                    op0=Alu.is_gt,
                )
                # causal mask inside the diagonal 128 columns
                nc.vector.tensor_tensor(
                    gj[:, 0:P], gj[:, 0:P], causal_sb, op=Alu.mult
                )
                # bit-pack matmuls: accumulate 16-wide qualifier masks into
                # every open t-chunk this plane tile touches
                for tcn in range(j // 4, NCH):
                    tlo, thi = max(c0, tcn * CW), (tcn + 1) * CW
                    if j == 0:
                        pk = psPK.tile([P, CW], f32, tag="pk",
                                       name=f"pk{b}_{tcn}")
                        pks[tcn] = pk
                    nc.tensor.matmul(
                        pks[tcn][:, tlo - tcn * CW : thi - tcn * CW],
                        lhsT=pw_sb[:, c0 : c0 + P],
                        rhs=gj[:, tlo - c0 : thi - c0],
                        start=(j == 0),
                        stop=(j == 4 * tcn + 3),
                    )
                # deferred chunk-close: decode chunk (j-7)//4 one chunk late
                # so the DVE never stalls on the ACT evacuation
                if (j + 1) % 4 == 0 and j >= 7:
                    chunk_close(b, (j - 7) // 4)

        def chunk_close(b, tcn):
            """Evacuate + decode chunk tcn of batch b: C16 -> F -> rc -> tau."""
            pks, rc, tauc = state[b]["pks"], state[b]["rc"], state[b]["tauc"]
            c1 = decp.tile([P, CW], f32, tag="c1", name=f"c1_{b}_{tcn}")
            nc.scalar.copy(c1, pks[tcn])
            mask = decp.tile([P, CW], f16, tag="mask", name=f"mask{b}_{tcn}")
            nc.vector.tensor_scalar(mask, c1, 0.0, None, op0=Alu.is_gt)
            eu = decp.tile([P, CW], u32, tag="eu", name=f"eu{b}_{tcn}")
            nc.vector.tensor_scalar(
                eu, c1.bitcast(u32), 23, None, op0=Alu.logical_shift_right
            )
            # F = (exponent + 16q - 126) * mask = (s_max within chunk q) + 1
            # (the uint32 exponent feeds the arithmetic op directly -- only
            # bitVec ops forbid the input dtype cast)
            ff = decp.tile([P, CW], f16, tag="ff", name=f"ff{b}_{tcn}")
            nc.vector.scalar_tensor_tensor(
                ff, eu, qb_sb, mask, op0=Alu.add, op1=Alu.mult
            )
            # cross-partition max via PE transpose + free-dim reduce
            pT = psT.tile([P, 4, P], f16, tag="pst", name=f"pT{b}_{tcn}")
            for m in range(4):
                nc.tensor.transpose(
                    pT[:, m, :], ff[:, m * P : (m + 1) * P], identh_sb
                )
            nc.vector.tensor_reduce(
                rc[:, 4 * tcn : 4 * tcn + 4], pT, axis=Ax.X, op=Alu.max
            )
            # tau = (rc > 0) ? (t + 1 - rc) : 0
            sl = slice(4 * tcn, 4 * tcn + 4)
            m01 = state[b]["m01"]
            td = state[b]["td"]
            nc.vector.tensor_scalar(m01[:, sl], rc[:, sl], 0.0, None, op0=Alu.is_gt)
            nc.vector.tensor_tensor(td[:, sl], t1_sb[:, sl], rc[:, sl],
                                    op=Alu.subtract)
            nc.vector.tensor_tensor(tauc[:, sl], td[:, sl], m01[:, sl],
                                    op=Alu.mult)
            # on-chip tau transport: duplicate each of the 4 tile-columns
            # (for the hi/lo weight rows), transpose to [8, P] at base 0 --
            # tau <= 2047 is exact in fp16
            tauc8 = decp.tile([P, 4, 2], f16, tag="tauc8",
                              name=f"tauc8_{b}_{tcn}")
            nc.vector.tensor_copy(tauc8[:, :, 0], tauc[:, sl])
            nc.vector.tensor_copy(tauc8[:, :, 1], tauc[:, sl])
            tauT_ps = psT.tile([8, P], f16, tag="pst", name=f"tauTps{b}_{tcn}")
            nc.tensor.transpose(
                tauT_ps, tauc8.rearrange("p a k -> p (a k)"), identh_sb
            )
            tauT8 = decp.tile([8, P], f16, tag="tauT8", bufs=4,
                              name=f"tauT8_{b}_{tcn}")
            nc.scalar.copy(tauT8, tauT_ps)
            state[b]["tauT8"][tcn] = tauT8

        def conv_chunk(b, tcn):
            """Conv + pe + tau matmuls, evacuation and output DMA for the 4
            token tiles of chunk tcn of batch b."""
            _, _, xTA = st[b]
            tauT8 = state[b]["tauT8"][tcn]
            if b == 0 and tcn == 0:
                # absorb the pe16 const-DMA wait before the first conv matmul
                nc.tensor.ldweights(pe16_sb[:, 0, 0:P])
            osb4 = outp.tile([P, 4, D], f32, tag="osb", name=f"osb{b}_{tcn}")
            for jj in range(4):
                i = 4 * tcn + jj
                ps = psW.tile([P, D], f32, tag="work", name=f"ps{b}_{i}")
                nc.tensor.matmul(
                    ps, lhsT=identh_sb, rhs=pe16_sb[:, i, :], start=True,
                    stop=False,
                )
                nc.tensor.matmul(
                    ps,
                    lhsT=xTA[:, i * P : (i + 1) * P],
                    rhs=wt01_sb,
                    start=False,
                    stop=False,
                )
                nc.tensor.matmul(
                    ps,
                    lhsT=xTA[C : 2 * C, i * P + 1 : (i + 1) * P + 1],
                    rhs=wt2_sb[C : 2 * C, :],
                    start=False,
                    stop=False,
                )
                nc.tensor.matmul(
                    ps,
                    lhsT=tauT8,
                    rhs=tcw8_sb[:, jj, :],
                    start=False,
                    stop=True,
                    tile_position=(0, 0),
                )
                nc.scalar.copy(osb4[:, jj, :], ps)
                if jj == 1:
                    # first half ships as soon as its two tiles are staged;
                    # two HWDGE rings so the packets interleave across SDMA
                    ov = out[b, tcn * CW : (tcn + 1) * CW, :].rearrange(
                        "(jj p) d -> p jj d", p=P
                    )
                    nc.sync.dma_start(ov[:, 0:2, :], osb4[:, 0:2, :])
            nc.scalar.dma_start(ov[:, 2:4, :], osb4[:, 2:4, :])

        state = {}
        for b in range(BLOC):
            rc = small.tile([P, NT], f32, tag="rc", name=f"rc{b}")
            m01 = small.tile([P, NT], f32, tag="m01", name=f"m01_{b}")
            td = small.tile([P, NT], f32, tag="td", name=f"td{b}")
            tauc = small.tile([P, NT], f32, tag="tauc", name=f"tauc{b}")
            state[b] = dict(pks={}, rc=rc, m01=m01, td=td, tauc=tauc,
                            tauT8={})

        # conv chunks slot into the plane j-loop as soon as their tau
        # transport can have landed, keeping the PE dense (HAM stays warm);
        # batch 0's plane starts before batch 1's phase A, and the pe16 load
        # dispatches after the startup-critical sync-ring traffic
        phase_a(0)
        phase_a(1)
        nc.scalar.dma_start(pe16_sb, pe16.rearrange("(i p) d -> p i d", p=P))
        plane_steps(0, range(0, 12))
        conv_chunk(0, 0)
        plane_steps(0, range(12, 16))
        conv_chunk(0, 1)
        chunk_close(0, 3)
        plane_steps(1, range(0, 4))
        conv_chunk(0, 2)
        plane_steps(1, range(4, 8))
        conv_chunk(0, 3)
        plane_steps(1, range(8, 12))
        conv_chunk(1, 0)
        plane_steps(1, range(12, 16))
        conv_chunk(1, 1)
        chunk_close(1, 3)
        conv_chunk(1, 2)
        conv_chunk(1, 3)


def build_bass():
    """Build the per-core Bass module (traced once, then bacc-compiled)."""
    import concourse.tile as tile
    from concourse import bacc, mybir

    f32 = mybir.dt.float32
    f16 = mybir.dt.float16

    nc = bacc.Bacc(
        "TRN2",
        target_bir_lowering=False,
        debug=False,
        enable_asserts=False,
        num_devices=NCORES,
    )
    aps = {}
    aps["xin"] = nc.dram_tensor("xin", (BLOC, S, C), f32, kind="ExternalInput").ap()
    aps["pe16"] = nc.dram_tensor("pe16", (S, D), f16, kind="ExternalInput").ap()
    aps["wt01"] = nc.dram_tensor("wt01", (P, D), f16, kind="ExternalInput").ap()
    aps["wt2"] = nc.dram_tensor("wt2", (C, D), f16, kind="ExternalInput").ap()
    aps["tcw8"] = nc.dram_tensor("tcw8", (8, NCH, D), f16, kind="ExternalInput").ap()
    aps["identh"] = nc.dram_tensor("identh", (P, P), f16, kind="ExternalInput").ap()
    aps["causal"] = nc.dram_tensor("causal", (P, P), f16, kind="ExternalInput").ap()
    aps["t1c"] = nc.dram_tensor("t1c", (P, NT), f32, kind="ExternalInput").ap()
    aps["pw"] = nc.dram_tensor("pw", (P, S), f16, kind="ExternalInput").ap()
    aps["selw"] = nc.dram_tensor("selw", (P, S), f16, kind="ExternalInput").ap()
    aps["qb"] = nc.dram_tensor("qb", (P, 1), f32, kind="ExternalInput").ap()
    aps["out"] = nc.dram_tensor("out", (BLOC, S, D), f32, kind="ExternalOutput").ap()

    with tile.TileContext(nc) as tc:
        _emit(tc, aps)
    nc.compile()
    return nc


def make_consts():
    """Host-side constant tensors (replicated params + index helpers)."""
    # positional embedding, matching the reference formula in fp32
    position = np.arange(S, dtype=np.float32)[:, None]
    div_term = np.exp(
        np.arange(0, D, 2, dtype=np.float32) * np.float32(-math.log(10000.0) / D)
    ).astype(np.float32)
    ang = (position * div_term).astype(np.float32)
    pe = np.zeros((S, D), dtype=np.float32)
    pe[:, 0::2] = np.sin(ang)
    pe[:, 1::2] = np.cos(ang)

    sl = np.arange(P, dtype=np.float32)[:, None]
    jj = np.arange(NT, dtype=np.float32)[None, :]
    # bit-pack matrix: PW[p, 128j + m] = 2^(p%16) if m == 8j + p//16
    pwm = np.zeros((P, S), dtype=np.float16)
    for j in range(NT):
        for p in range(P):
            pwm[p, 128 * j + 8 * j + p // 16] = np.float16(2.0 ** (p % 16))
    # throw-broadcast selector: column block j selects the 3 split components
    # of tile j (rows 8j..8j+2) with weights (1, 2^-12, 2^-12)
    selw = np.zeros((P, S), dtype=np.float16)
    for j in range(NT):
        for k, w in enumerate((1.0, 2.0**-12, 2.0**-12)):
            selw[8 * j + k, 128 * j : 128 * (j + 1)] = np.float16(w)
    consts = {
        "identh": np.eye(P, dtype=np.float16),
        "causal": (np.arange(P)[:, None] < np.arange(P)[None, :]).astype(np.float16),
        "t1c": (jj * P + sl + 1.0).astype(np.float32),
        "pw": pwm,
        "selw": selw,
        "qb": (16.0 * np.arange(P, dtype=np.float32) - 126.0)[:, None],
    }
    return pe, consts


def make_shared_inputs(conv_w, tc_w, tc_b):
    pe, consts = make_consts()
    pe_b = (pe + np.asarray(tc_b, np.float32)[None, :]).astype(np.float32)
    # conv weights, channel-major per tap: wk[c, d] = conv_w[d, c, k]
    wt = np.transpose(np.asarray(conv_w, np.float32), (2, 1, 0))  # (k, c, d)
    wt01 = np.concatenate([wt[0], wt[1]], axis=0).astype(np.float16)  # (128, D)
    wt2 = wt[2].astype(np.float16)  # (64, D)
    # tc_w split into fp16 hi+lo (tau amplifies errors by up to 2048);
    # tcw8[2jj, jj] = w_hi, tcw8[2jj+1, jj] = w_lo selects tile-in-chunk jj
    w = np.asarray(tc_w, np.float32)[:, 0]
    w_hi = w.astype(np.float16)
    w_lo = (w - w_hi.astype(np.float32)).astype(np.float16)
    tcw8 = np.zeros((8, NCH, D), dtype=np.float16)
    for jj in range(NCH):
        tcw8[2 * jj, jj, :] = w_hi
        tcw8[2 * jj + 1, jj, :] = w_lo
    return {
        "pe16": pe_b.astype(np.float16),
        "wt01": np.ascontiguousarray(wt01),
        "wt2": np.ascontiguousarray(wt2),
        "tcw8": tcw8,
        **{k: np.ascontiguousarray(v) for k, v in consts.items()},
    }


_BUILD_CACHE = {}


def _install_ntff_hook():
    """The agent image's antenv lacks axon_hooks; synthesize it from the
    boot module's ctypes implementation so trace=True works under axon."""
    import sys as _sys
    import types

    if "antenv.axon_hooks" in _sys.modules:
        return
    try:
        from trn_agent_boot.trn_boot import _ntff_profile_via_ctypes

        hook = _ntff_profile_via_ctypes("/opt/axon/libaxon_pjrt.so")
        m = types.ModuleType("antenv.axon_hooks")
        m.get_axon_ntff_profile_hook = lambda: hook
        _sys.modules["antenv.axon_hooks"] = m
    except Exception as e:  # degrade to no-trace
        print("[kernel] ntff hook install failed:", e)


def kernel(x, conv_w, tc_w, tc_b):
    x = np.ascontiguousarray(np.asarray(x, dtype=np.float32))
    conv_w = np.asarray(conv_w, dtype=np.float32)
    tc_w = np.asarray(tc_w, dtype=np.float32)
    tc_b = np.asarray(tc_b, dtype=np.float32)
    assert x.shape == (B, S, C), x.shape

    from concourse.bass_utils import run_bass_kernel_spmd

    if "nc" not in _BUILD_CACHE:
        _BUILD_CACHE["nc"] = build_bass()
    nc = _BUILD_CACHE["nc"]

    shared = make_shared_inputs(conv_w, tc_w, tc_b)
    in_maps = []
    for c in range(NCORES):
        m = dict(shared)
        m["xin"] = np.ascontiguousarray(x[c * BLOC : (c + 1) * BLOC])
        in_maps.append(m)

    trace = bool(int(os.environ.get("KERNEL_TRACE", "0")))
    if trace:
        _install_ntff_hook()
    res = run_bass_kernel_spmd(
        nc, in_maps, core_ids=list(range(NCORES)), trace=trace, trace_cores=[0]
    )
    if trace and res.exec_time_ns is not None:
        print(
            f"[kernel] HW exec time: {res.exec_time_ns} ns "
            f"(mean {res.mean_exec_time_ns} ns)"
        )
        kernel.last_exec_time_ns = res.exec_time_ns
        kernel.last_trace = res.instructions_and_trace
    out = np.concatenate([r["out"] for r in res.results], axis=0)
    return out


if __name__ == "__main__":
    build_bass()
    print("build ok")


# revision 58
# speedup vs baseline: 1.2648x; 1.1666x over previous
"""Trainium2 Bass kernel for nn_DataEmbedding_v2 (circular conv1d + sinusoidal PE
+ causal threshold-scan "tc" embedding).

Contract: kernel(**inputs) takes FULL inputs (x:(16,2048,64) f32, conv_w:(512,64,3),
tc_w:(512,1), tc_b:(512,)) and returns the FULL (16,2048,512) f32 output.
Internally: batch-sharded data-parallel across 8 NeuronCores (2 batches/core),
params replicated.

v2 structure (vs the 150us baseline): the causal threshold scan's argmax fold
moved off the vector engine onto the PE as bit-pack matmuls: a constant
power-of-two matrix PW packs each 16-token s-chunk's qualifier mask into an
integer C16 = sum(cond * 2^r) accumulated in PSUM per 512-token t-chunk; the
within-chunk argmax is recovered elementwise with an exponent-extraction
(bitcast >> 23) instead of a cross-partition fold.  Everything streams per
512-token chunk (pack -> decode -> tau roundtrip -> conv -> output DMA), with
batch 1's plane interleaved into batch 0's conv so the PE stays dense/warm.

Numerics: the tc threshold compare runs in exact fp32 (bit-matching the
reference's rounding); the bit-pack path is exact (integer C16 <= 65535 in
f32 PSUM, tau <= 2048 exact in fp16); conv weights/activations and pe run in
fp16 on the PE; tau*tc_w uses an fp16 hi+lo split of tc_w.
"""

import math
import os
import sys

sys.path.insert(0, "/opt/trn_rl_repo")

import numpy as np

B, S, C, D = 16, 2048, 64, 512
NCORES = 8
BLOC = B // NCORES  # batches per core
P = 128
NT = S // P  # 16 tiles of 128 tokens
NCH = 4  # t-chunks
CW = S // NCH  # 512 tokens per chunk
ETA = 0.3
EPS = 1e-08
# gpsimd tensor ops measured ~12.6ns/col on HW (6-25x slower than DVE) and
# degrade concurrent DVE throughput via SBUF port contention -- keep the
# plane entirely on the vector engine.
GP_J = ()


def _emit(tc, aps):
    """Emit the per-core Tile kernel. aps: dict of DRAM APs."""
    from contextlib import ExitStack

    from concourse import mybir

    f32 = mybir.dt.float32
    f16 = mybir.dt.float16
    u32 = mybir.dt.uint32
    Alu = mybir.AluOpType
    Ax = mybir.AxisListType

    nc = tc.nc
    xin, pe16, wt01, wt2 = aps["xin"], aps["pe16"], aps["wt01"], aps["wt2"]
    identh, causal = aps["identh"], aps["causal"]
    t1c, pw, qb, out = aps["t1c"], aps["pw"], aps["qb"], aps["out"]

    with ExitStack() as ctx:
        singles = ctx.enter_context(tc.tile_pool(name="singles", bufs=1))
        xpool = ctx.enter_context(tc.tile_pool(name="xpool", bufs=2))
        small = ctx.enter_context(tc.tile_pool(name="small", bufs=2))
        rowpool = ctx.enter_context(tc.tile_pool(name="rowpool", bufs=2))
        xtp = ctx.enter_context(tc.tile_pool(name="xtp", bufs=2))
        gjp = ctx.enter_context(tc.tile_pool(name="gjp", bufs=3))
        decp = ctx.enter_context(tc.tile_pool(name="decp", bufs=2))
        outp = ctx.enter_context(tc.tile_pool(name="outp", bufs=2))
        psW = ctx.enter_context(tc.tile_pool(name="psW", bufs=3, space="PSUM"))
        psPK = ctx.enter_context(tc.tile_pool(name="psPK", bufs=4, space="PSUM"))
        psT = ctx.enter_context(tc.tile_pool(name="psT", bufs=1, space="PSUM"))

        # ---- x loads first (sync ring), then small consts (sync), pe16 on
        # the scalar ring so the big PE table doesn't block the critical path.
        xins = {}
        for b in range(BLOC):
            xin_sb = xpool.tile([P, NT, C], f32, tag="xin", name=f"xin_sb{b}")
            nc.sync.dma_start(xin_sb, xin[b].rearrange("(j p) c -> p j c", p=P))
            xins[b] = xin_sb
        identh_sb = singles.tile([P, P], f16)
        nc.sync.dma_start(identh_sb, identh)
        causal_sb = singles.tile([P, P], f16)
        nc.sync.dma_start(causal_sb, causal)
        selw_sb = singles.tile([P, S], f16)
        nc.sync.dma_start(selw_sb, aps["selw"])
        t1_sb = singles.tile([P, NT], f32)
        nc.sync.dma_start(t1_sb, t1c)
        qb_sb = singles.tile([P, 1], f32)
        nc.sync.dma_start(qb_sb, qb)
        wt01_sb = singles.tile([P, D], f16)
        nc.sync.dma_start(wt01_sb, wt01)
        wt2_sb = singles.tile([P, D], f16)
        nc.sync.dma_start(wt2_sb[C : 2 * C, :], wt2)
        tcw8_sb = singles.tile([8, NCH, D], f16)
        nc.sync.dma_start(tcw8_sb, aps["tcw8"])
        pw_sb = singles.tile([P, S], f16)
        nc.sync.dma_start(pw_sb, pw)

        # ---- PE priming: absorb each const-DMA wait on a dedicated PE op so
        # later matmuls never carry more than one sync wait.  Only the consts
        # needed by the throw broadcast prime here; the plane/conv weights
        # prime later so they don't stall the startup-critical PE chain.
        prime_h = psT.tile([P, P], f16, tag="pst")
        nc.tensor.transpose(prime_h, identh_sb, identh_sb)
        dumps = singles.tile([P, 4], f32)
        nc.scalar.copy(dumps[:, 0:1], prime_h[:, 0:1])
        nc.tensor.ldweights(selw_sb[:, 0:P])
        nc.tensor.ldweights(wt01_sb[:, 0:P])
        nc.tensor.ldweights(wt2_sb[C : 2 * C, 0:P], tile_position=(64, 0))
        nc.tensor.ldweights(tcw8_sb[:, 0, 0:P])
        nc.tensor.ldweights(pw_sb[:, 0:P])

        # ---- Phase A (per batch): norms/thresh/broadcast + xTA build ----
        st = {}

        def phase_a(b):
            xin_sb = xins[b]
            # norms (L1 over channels), two-level sum (bit-matches reference)
            r8 = small.tile([P, NT, 8], f32, tag="r8", name=f"r8_{b}")
            nc.vector.tensor_reduce(
                r8,
                xin_sb.rearrange("p j (a b) -> p j a b", b=8),
                axis=Ax.X,
                op=Alu.add,
                apply_absolute_value=True,
            )
            normc = small.tile([P, NT], f32, tag="normc", name=f"normc{b}")
            nc.vector.tensor_reduce(normc, r8, axis=Ax.X, op=Alu.add)
            # thresh = (norms + EPS) * 0.7 (exact fp32 rounding order of ref)
            thc = small.tile([P, NT], f32, tag="thc", name=f"thc{b}")
            nc.vector.tensor_scalar(
                thc, normc, float(EPS), float(1.0 - ETA), op0=Alu.add, op1=Alu.mult
            )
            # thresh broadcast over partitions: short DRAM roundtrip for the
            # s-contiguous row, then a K=1 ones-matmul fans it out on the PE
            # (the 1MB partition_broadcast DMA costs ~30us; this costs ~5us)
            # thresh broadcast over partitions: exact 3-way f16 split
            # (hi + mid/4096 + lo/4096 == thc bit-exact; the scaling keeps
            # the f16 components normal), short DRAM roundtrip for the
            # s-contiguous rows, then a K=3 ones-matmul fans out on the PE.
            # (partition_broadcast DMA of the 1MB plane costs ~30us serial.)
            th3 = small.tile([P, NT, 8], f16, tag="th3", name=f"th3_{b}")
            nc.vector.memset(th3, 0)
            r1 = small.tile([P, NT], f32, tag="r1", name=f"r1_{b}")
            r2n = small.tile([P, NT], f32, tag="r2n", name=f"r2n_{b}")
            nc.vector.tensor_copy(th3[:, :, 0], thc)
            nc.vector.tensor_tensor(r1, thc, th3[:, :, 0], op=Alu.subtract)
            nc.vector.tensor_scalar(th3[:, :, 1], r1, 4096.0, None, op0=Alu.mult)
            nc.vector.scalar_tensor_tensor(
                r2n, th3[:, :, 1], float(2.0**-12), r1,
                op0=Alu.mult, op1=Alu.subtract,
            )
            nc.vector.tensor_scalar(th3[:, :, 2], r2n, -4096.0, None, op0=Alu.mult)
            # on-chip permutation: transpose [P, (j k)] -> [(j k), P] puts
            # tile j's three components at partitions 8j..8j+2 (k=3..7 zero);
            # the selw matmul (3 nonzero rows per output tile) fans them out
            th8T_ps = psT.tile([P, P], f16, tag="pst", name=f"th8Tps{b}")
            nc.tensor.transpose(
                th8T_ps, th3.rearrange("p j k -> p (j k)"), identh_sb
            )
            th8T = rowpool.tile([P, P], f16, tag="throwrow", name=f"th8T{b}")
            nc.scalar.copy(th8T, th8T_ps)
            throw = rowpool.tile([P, S], f32, tag="throw", name=f"throw{b}")
            for g in range(NCH):
                thps = psW.tile([P, CW], f32, tag="work", name=f"thps{b}_{g}")
                for m in range(4):
                    j = 4 * g + m
                    nc.tensor.matmul(
                        thps[:, m * P : (m + 1) * P],
                        lhsT=selw_sb[:, j * P : (j + 1) * P],
                        rhs=th8T,
                        start=(m == 0),
                        stop=(m == 3),
                    )
                nc.scalar.copy(throw[:, g * CW : (g + 1) * CW], thps)

            # xTA: fp16 x, channel-major, two stacked tap views.
            # 4 transposes share one PSUM bank, evacuated by 2 wide ACT copies.
            xin16 = xpool.tile([P, NT, C], f16, tag="xin16", name=f"xin16_{b}")
            nc.vector.tensor_copy(xin16, xin_sb)
            nc.tensor.ldweights(xin16[:, 0, :])  # absorb DVE wait for PE
            xTA = xtp.tile([P, S + 1], f16, tag="xta", name=f"xTA{b}")
            for g in range(4):
                xtg = psW.tile([C, 4, P], f16, tag="work", name=f"xtg{b}_{g}")
                for m in range(4):
                    nc.tensor.transpose(
                        xtg[:, m, :], xin16[:, 4 * g + m, :], identh_sb
                    )
                c0 = g * 4 * P
                nc.scalar.copy(
                    xTA[0:C, 1 + c0 : 1 + c0 + 4 * P],
                    xtg.rearrange("p m q -> p (m q)"),
                )
                nc.scalar.copy(
                    xTA[C : 2 * C, c0 : c0 + 4 * P],
                    xtg.rearrange("p m q -> p (m q)"),
                )
            nc.scalar.copy(xTA[0:C, 0:1], xTA[0:C, S : S + 1])
            nc.scalar.copy(xTA[C : 2 * C, S : S + 1], xTA[C : 2 * C, 0:1])
            st[b] = (normc, throw, xTA)

        pe16_sb = singles.tile([P, NT, D], f16)

        # ---- streamed plane + pack + decode + tau + conv ----
        def plane_steps(b, jlist):
            """Emit plane tiles (compare + causal + bit-pack matmuls) for the
            given j's of batch b, plus deferred chunk-close decode work."""
            normc, throw, _ = st[b]
            pks, rcs, taucs = state[b]["pks"], state[b]["rc"], state[b]["tauc"]
            for j in jlist:
                c0 = j * P
                w = S - c0
                gj = gjp.tile([P, S], f16, tag="gj", name=f"gj{b}_{j}")
                # cond[s, t] = thresh[t] > norms[s]  (exact fp32 compare)
                nc.vector.tensor_scalar(
                    gj[:, 0:w], throw[:, c0:S], normc[:, j : j + 1], None,
                    op0=Alu.is_gt,
                )
                # causal mask inside the diagonal 128 columns
                nc.vector.tensor_tensor(
                    gj[:, 0:P], gj[:, 0:P], causal_sb, op=Alu.mult
                )
                # bit-pack matmuls: accumulate 16-wide qualifier masks into
                # every open t-chunk this plane tile touches
                for tcn in range(j // 4, NCH):
                    tlo, thi = max(c0, tcn * CW), (tcn + 1) * CW
                    if j == 0:
                        pk = psPK.tile([P, CW], f32, tag="pk",
                                       name=f"pk{b}_{tcn}")
                        pks[tcn] = pk
                    nc.tensor.matmul(
                        pks[tcn][:, tlo - tcn * CW : thi - tcn * CW],
                        lhsT=pw_sb[:, c0 : c0 + P],
                        rhs=gj[:, tlo - c0 : thi - c0],
                        start=(j == 0),
                        stop=(j == 4 * tcn + 3),
                    )
                # deferred chunk-close: decode chunk (j-7)//4 one chunk late
                # so the DVE never stalls on the ACT evacuation
                if (j + 1) % 4 == 0 and j >= 7:
                    chunk_close(b, (j - 7) // 4)

        def chunk_close(b, tcn):
            """Evacuate + decode chunk tcn of batch b: C16 -> F -> rc -> tau."""
            pks, rc, tauc = state[b]["pks"], state[b]["rc"], state[b]["tauc"]
            c1 = decp.tile([P, CW], f32, tag="c1", name=f"c1_{b}_{tcn}")
            nc.scalar.copy(c1, pks[tcn])
            mask = decp.tile([P, CW], f16, tag="mask", name=f"mask{b}_{tcn}")
            nc.vector.tensor_scalar(mask, c1, 0.0, None, op0=Alu.is_gt)
            eu = decp.tile([P, CW], u32, tag="eu", name=f"eu{b}_{tcn}")
            nc.vector.tensor_scalar(
                eu, c1.bitcast(u32), 23, None, op0=Alu.logical_shift_right
            )
            # F = (exponent + 16q - 126) * mask = (s_max within chunk q) + 1
            # (the uint32 exponent feeds the arithmetic op directly -- only
            # bitVec ops forbid the input dtype cast)
            ff = decp.tile([P, CW], f16, tag="ff", name=f"ff{b}_{tcn}")
            nc.vector.scalar_tensor_tensor(
                ff, eu, qb_sb, mask, op0=Alu.add, op1=Alu.mult
            )
            # cross-partition max via PE transpose + free-dim reduce
            pT = psT.tile([P, 4, P], f16, tag="pst", name=f"pT{b}_{tcn}")
            for m in range(4):
                nc.tensor.transpose(
                    pT[:, m, :], ff[:, m * P : (m + 1) * P], identh_sb
                )
            nc.vector.tensor_reduce(
                rc[:, 4 * tcn : 4 * tcn + 4], pT, axis=Ax.X, op=Alu.max
            )
            # tau = (rc > 0) ? (t + 1 - rc) : 0
            sl = slice(4 * tcn, 4 * tcn + 4)
            m01 = state[b]["m01"]
            td = state[b]["td"]
            nc.vector.tensor_scalar(m01[:, sl], rc[:, sl], 0.0, None, op0=Alu.is_gt)
            nc.vector.tensor_tensor(td[:, sl], t1_sb[:, sl], rc[:, sl],
                                    op=Alu.subtract)
            nc.vector.tensor_tensor(tauc[:, sl], td[:, sl], m01[:, sl],
                                    op=Alu.mult)
            # on-chip tau transport: duplicate each of the 4 tile-columns
            # (for the hi/lo weight rows), transpose to [8, P] at base 0 --
            # tau <= 2047 is exact in fp16
            tauc8 = decp.tile([P, 4, 2], f16, tag="tauc8",
                              name=f"tauc8_{b}_{tcn}")
            nc.vector.tensor_copy(tauc8[:, :, 0], tauc[:, sl])
            nc.vector.tensor_copy(tauc8[:, :, 1], tauc[:, sl])
            tauT_ps = psT.tile([8, P], f16, tag="pst", name=f"tauTps{b}_{tcn}")
            nc.tensor.transpose(
                tauT_ps, tauc8.rearrange("p a k -> p (a k)"), identh_sb
            )
            tauT8 = decp.tile([8, P], f16, tag="tauT8", bufs=4,
                              name=f"tauT8_{b}_{tcn}")
            nc.scalar.copy(tauT8, tauT_ps)
            state[b]["tauT8"][tcn] = tauT8

        def conv_chunk(b, tcn):
            """Conv + pe + tau matmuls, evacuation and output DMA for the 4
            token tiles of chunk tcn of batch b."""
            _, _, xTA = st[b]
            tauT8 = state[b]["tauT8"][tcn]
            if b == 0 and tcn == 0:
                # absorb the pe16 const-DMA wait before the first conv matmul
                nc.tensor.ldweights(pe16_sb[:, 0, 0:P])
            osb4 = outp.tile([P, 4, D], f32, tag="osb", name=f"osb{b}_{tcn}")
            for jj in range(4):
                i = 4 * tcn + jj
                ps = psW.tile([P, D], f32, tag="work", name=f"ps{b}_{i}")
                nc.tensor.matmul(
                    ps, lhsT=identh_sb, rhs=pe16_sb[:, i, :], start=True,
                    stop=False,
                )
                nc.tensor.matmul(
                    ps,
                    lhsT=xTA[:, i * P : (i + 1) * P],
                    rhs=wt01_sb,
                    start=False,
                    stop=False,
                )
                nc.tensor.matmul(
                    ps,
                    lhsT=xTA[C : 2 * C, i * P + 1 : (i + 1) * P + 1],
                    rhs=wt2_sb[C : 2 * C, :],
                    start=False,
                    stop=False,
                )
                nc.tensor.matmul(
                    ps,
                    lhsT=tauT8,
                    rhs=tcw8_sb[:, jj, :],
                    start=False,
                    stop=True,
                    tile_position=(0, 0),
                )
                nc.scalar.copy(osb4[:, jj, :], ps)
            # two half-DMAs on both HWDGE rings: halves the per-chunk
            # completion latency (packets interleave across SDMA engines)
            ov = out[b, tcn * CW : (tcn + 1) * CW, :].rearrange(
                "(jj p) d -> p jj d", p=P
            )
            nc.sync.dma_start(ov[:, 0:2, :], osb4[:, 0:2, :])
            nc.scalar.dma_start(ov[:, 2:4, :], osb4[:, 2:4, :])

        state = {}
        for b in range(BLOC):
            rc = small.tile([P, NT], f32, tag="rc", name=f"rc{b}")
            m01 = small.tile([P, NT], f32, tag="m01", name=f"m01_{b}")
            td = small.tile([P, NT], f32, tag="td", name=f"td{b}")
            tauc = small.tile([P, NT], f32, tag="tauc", name=f"tauc{b}")
            state[b] = dict(pks={}, rc=rc, m01=m01, td=td, tauc=tauc,
                            tauT8={})

        # conv chunks slot into the plane j-loop as soon as their tau
        # transport can have landed, keeping the PE dense (HAM stays warm);
        # batch 0's plane starts before batch 1's phase A, and the pe16 load
        # dispatches after the startup-critical sync-ring traffic
        phase_a(0)
        phase_a(1)
        nc.scalar.dma_start(pe16_sb, pe16.rearrange("(i p) d -> p i d", p=P))
        plane_steps(0, range(0, 12))
        conv_chunk(0, 0)
        plane_steps(0, range(12, 16))
        conv_chunk(0, 1)
        chunk_close(0, 3)
        plane_steps(1, range(0, 4))
        conv_chunk(0, 2)
        plane_steps(1, range(4, 8))
        conv_chunk(0, 3)
        plane_steps(1, range(8, 12))
        conv_chunk(1, 0)
        plane_steps(1, range(12, 16))
        conv_chunk(1, 1)
        chunk_close(1, 3)
        conv_chunk(1, 2)
        conv_chunk(1, 3)


def build_bass():
    """Build the per-core Bass module (traced once, then bacc-compiled)."""
    import concourse.tile as tile
    from concourse import bacc, mybir

    f32 = mybir.dt.float32
    f16 = mybir.dt.float16

    nc = bacc.Bacc(
        "TRN2",
        target_bir_lowering=False,
        debug=False,
        enable_asserts=False,
        num_devices=NCORES,
    )
    aps = {}
    aps["xin"] = nc.dram_tensor("xin", (BLOC, S, C), f32, kind="ExternalInput").ap()
    aps["pe16"] = nc.dram_tensor("pe16", (S, D), f16, kind="ExternalInput").ap()
    aps["wt01"] = nc.dram_tensor("wt01", (P, D), f16, kind="ExternalInput").ap()
    aps["wt2"] = nc.dram_tensor("wt2", (C, D), f16, kind="ExternalInput").ap()
    aps["tcw8"] = nc.dram_tensor("tcw8", (8, NCH, D), f16, kind="ExternalInput").ap()
    aps["identh"] = nc.dram_tensor("identh", (P, P), f16, kind="ExternalInput").ap()
    aps["causal"] = nc.dram_tensor("causal", (P, P), f16, kind="ExternalInput").ap()
    aps["t1c"] = nc.dram_tensor("t1c", (P, NT), f32, kind="ExternalInput").ap()
    aps["pw"] = nc.dram_tensor("pw", (P, S), f16, kind="ExternalInput").ap()
    aps["selw"] = nc.dram_tensor("selw", (P, S), f16, kind="ExternalInput").ap()
    aps["qb"] = nc.dram_tensor("qb", (P, 1), f32, kind="ExternalInput").ap()
    aps["out"] = nc.dram_tensor("out", (BLOC, S, D), f32, kind="ExternalOutput").ap()

    with tile.TileContext(nc) as tc:
        _emit(tc, aps)
    nc.compile()
    return nc


def make_consts():
    """Host-side constant tensors (replicated params + index helpers)."""
    # positional embedding, matching the reference formula in fp32
    position = np.arange(S, dtype=np.float32)[:, None]
    div_term = np.exp(
        np.arange(0, D, 2, dtype=np.float32) * np.float32(-math.log(10000.0) / D)
    ).astype(np.float32)
    ang = (position * div_term).astype(np.float32)
    pe = np.zeros((S, D), dtype=np.float32)
    pe[:, 0::2] = np.sin(ang)
    pe[:, 1::2] = np.cos(ang)

    sl = np.arange(P, dtype=np.float32)[:, None]
    jj = np.arange(NT, dtype=np.float32)[None, :]
    # bit-pack matrix: PW[p, 128j + m] = 2^(p%16) if m == 8j + p//16
    pwm = np.zeros((P, S), dtype=np.float16)
    for j in range(NT):
        for p in range(P):
            pwm[p, 128 * j + 8 * j + p // 16] = np.float16(2.0 ** (p % 16))
    # throw-broadcast selector: column block j selects the 3 split components
    # of tile j (rows 8j..8j+2) with weights (1, 2^-12, 2^-12)
    selw = np.zeros((P, S), dtype=np.float16)
    for j in range(NT):
        for k, w in enumerate((1.0, 2.0**-12, 2.0**-12)):
            selw[8 * j + k, 128 * j : 128 * (j + 1)] = np.float16(w)
    consts = {
        "identh": np.eye(P, dtype=np.float16),
        "causal": (np.arange(P)[:, None] < np.arange(P)[None, :]).astype(np.float16),
        "t1c": (jj * P + sl + 1.0).astype(np.float32),
        "pw": pwm,
        "selw": selw,
        "qb": (16.0 * np.arange(P, dtype=np.float32) - 126.0)[:, None],
    }
    return pe, consts


def make_shared_inputs(conv_w, tc_w, tc_b):
    pe, consts = make_consts()
    pe_b = (pe + np.asarray(tc_b, np.float32)[None, :]).astype(np.float32)
    # conv weights, channel-major per tap: wk[c, d] = conv_w[d, c, k]
    wt = np.transpose(np.asarray(conv_w, np.float32), (2, 1, 0))  # (k, c, d)
    wt01 = np.concatenate([wt[0], wt[1]], axis=0).astype(np.float16)  # (128, D)
    wt2 = wt[2].astype(np.float16)  # (64, D)
    # tc_w split into fp16 hi+lo (tau amplifies errors by up to 2048);
    # tcw8[2jj, jj] = w_hi, tcw8[2jj+1, jj] = w_lo selects tile-in-chunk jj
    w = np.asarray(tc_w, np.float32)[:, 0]
    w_hi = w.astype(np.float16)
    w_lo = (w - w_hi.astype(np.float32)).astype(np.float16)
    tcw8 = np.zeros((8, NCH, D), dtype=np.float16)
    for jj in range(NCH):
        tcw8[2 * jj, jj, :] = w_hi
        tcw8[2 * jj + 1, jj, :] = w_lo
    return {
        "pe16": pe_b.astype(np.float16),
        "wt01": np.ascontiguousarray(wt01),
        "wt2": np.ascontiguousarray(wt2),
        "tcw8": tcw8,
        **{k: np.ascontiguousarray(v) for k, v in consts.items()},
    }


_BUILD_CACHE = {}


def _install_ntff_hook():
    """The agent image's antenv lacks axon_hooks; synthesize it from the
    boot module's ctypes implementation so trace=True works under axon."""
    import sys as _sys
    import types

    if "antenv.axon_hooks" in _sys.modules:
        return
    try:
        from trn_agent_boot.trn_boot import _ntff_profile_via_ctypes

        hook = _ntff_profile_via_ctypes("/opt/axon/libaxon_pjrt.so")
        m = types.ModuleType("antenv.axon_hooks")
        m.get_axon_ntff_profile_hook = lambda: hook
        _sys.modules["antenv.axon_hooks"] = m
    except Exception as e:  # degrade to no-trace
        print("[kernel] ntff hook install failed:", e)


def kernel(x, conv_w, tc_w, tc_b):
    x = np.ascontiguousarray(np.asarray(x, dtype=np.float32))
    conv_w = np.asarray(conv_w, dtype=np.float32)
    tc_w = np.asarray(tc_w, dtype=np.float32)
    tc_b = np.asarray(tc_b, dtype=np.float32)
    assert x.shape == (B, S, C), x.shape

    from concourse.bass_utils import run_bass_kernel_spmd

    if "nc" not in _BUILD_CACHE:
        _BUILD_CACHE["nc"] = build_bass()
    nc = _BUILD_CACHE["nc"]

    shared = make_shared_inputs(conv_w, tc_w, tc_b)
    in_maps = []
    for c in range(NCORES):
        m = dict(shared)
        m["xin"] = np.ascontiguousarray(x[c * BLOC : (c + 1) * BLOC])
        in_maps.append(m)

    trace = bool(int(os.environ.get("KERNEL_TRACE", "0")))
    if trace:
        _install_ntff_hook()
    res = run_bass_kernel_spmd(
        nc, in_maps, core_ids=list(range(NCORES)), trace=trace, trace_cores=[0]
    )
    if trace and res.exec_time_ns is not None:
        print(
            f"[kernel] HW exec time: {res.exec_time_ns} ns "
            f"(mean {res.mean_exec_time_ns} ns)"
        )
        kernel.last_exec_time_ns = res.exec_time_ns
        kernel.last_trace = res.instructions_and_trace
    out = np.concatenate([r["out"] for r in res.results], axis=0)
    return out


if __name__ == "__main__":
    build_bass()
    print("build ok")


# revision 59
# speedup vs baseline: 1.2911x; 1.0207x over previous
"""Trainium2 Bass kernel for nn_DataEmbedding_v2 (circular conv1d + sinusoidal PE
+ causal threshold-scan "tc" embedding).

Contract: kernel(**inputs) takes FULL inputs (x:(16,2048,64) f32, conv_w:(512,64,3),
tc_w:(512,1), tc_b:(512,)) and returns the FULL (16,2048,512) f32 output.
Internally: batch-sharded data-parallel across 8 NeuronCores (2 batches/core),
params replicated.

v2 structure (vs the 150us baseline): the causal threshold scan's argmax fold
moved off the vector engine onto the PE as bit-pack matmuls: a constant
power-of-two matrix PW packs each 16-token s-chunk's qualifier mask into an
integer C16 = sum(cond * 2^r) accumulated in PSUM per 512-token t-chunk; the
within-chunk argmax is recovered elementwise with an exponent-extraction
(bitcast >> 23) instead of a cross-partition fold.  Everything streams per
512-token chunk (pack -> decode -> tau roundtrip -> conv -> output DMA), with
batch 1's plane interleaved into batch 0's conv so the PE stays dense/warm.

Numerics: the tc threshold compare runs in exact fp32 (bit-matching the
reference's rounding); the bit-pack path is exact (integer C16 <= 65535 in
f32 PSUM, tau <= 2048 exact in fp16); conv weights/activations and pe run in
fp16 on the PE; tau*tc_w uses an fp16 hi+lo split of tc_w.
"""

import math
import os
import sys

sys.path.insert(0, "/opt/trn_rl_repo")

import numpy as np

B, S, C, D = 16, 2048, 64, 512
NCORES = 8
BLOC = B // NCORES  # batches per core
P = 128
NT = S // P  # 16 tiles of 128 tokens
NCH = 4  # t-chunks
CW = S // NCH  # 512 tokens per chunk
ETA = 0.3
EPS = 1e-08
# gpsimd tensor ops measured ~12.6ns/col on HW (6-25x slower than DVE) and
# degrade concurrent DVE throughput via SBUF port contention -- keep the
# plane entirely on the vector engine.
GP_J = ()


def _emit(tc, aps):
    """Emit the per-core Tile kernel. aps: dict of DRAM APs."""
    from contextlib import ExitStack

    from concourse import mybir

    f32 = mybir.dt.float32
    f16 = mybir.dt.float16
    u32 = mybir.dt.uint32
    Alu = mybir.AluOpType
    Ax = mybir.AxisListType

    nc = tc.nc
    xin, pe16, wt01, wt2 = aps["xin"], aps["pe16"], aps["wt01"], aps["wt2"]
    identh, causal = aps["identh"], aps["causal"]
    t1c, pw, qb, out = aps["t1c"], aps["pw"], aps["qb"], aps["out"]

    with ExitStack() as ctx:
        singles = ctx.enter_context(tc.tile_pool(name="singles", bufs=1))
        xpool = ctx.enter_context(tc.tile_pool(name="xpool", bufs=2))
        small = ctx.enter_context(tc.tile_pool(name="small", bufs=2))
        rowpool = ctx.enter_context(tc.tile_pool(name="rowpool", bufs=2))
        xtp = ctx.enter_context(tc.tile_pool(name="xtp", bufs=2))
        gjp = ctx.enter_context(tc.tile_pool(name="gjp", bufs=3))
        decp = ctx.enter_context(tc.tile_pool(name="decp", bufs=2))
        outp = ctx.enter_context(tc.tile_pool(name="outp", bufs=2))
        psW = ctx.enter_context(tc.tile_pool(name="psW", bufs=3, space="PSUM"))
        psPK = ctx.enter_context(tc.tile_pool(name="psPK", bufs=4, space="PSUM"))
        psT = ctx.enter_context(tc.tile_pool(name="psT", bufs=1, space="PSUM"))

        # ---- x loads first (sync ring), then small consts (sync), pe16 on
        # the scalar ring so the big PE table doesn't block the critical path.
        # selw + identh lead the sync ring: they gate the PE priming chain
        # (throw broadcast) which gates the plane start
        identh_sb = singles.tile([P, P], f16)
        nc.sync.dma_start(identh_sb, identh)
        selw_sb = singles.tile([P, S], f16)
        nc.sync.dma_start(selw_sb, aps["selw"])
        xins = {}
        for b in range(BLOC):
            xin_sb = xpool.tile([P, NT, C], f32, tag="xin", name=f"xin_sb{b}")
            nc.sync.dma_start(xin_sb, xin[b].rearrange("(j p) c -> p j c", p=P))
            xins[b] = xin_sb
        causal_sb = singles.tile([P, P], f16)
        nc.sync.dma_start(causal_sb, causal)
        t1_sb = singles.tile([P, NT], f32)
        nc.sync.dma_start(t1_sb, t1c)
        qb_sb = singles.tile([P, 1], f32)
        nc.sync.dma_start(qb_sb, qb)
        wt01_sb = singles.tile([P, D], f16)
        nc.sync.dma_start(wt01_sb, wt01)
        wt2_sb = singles.tile([P, D], f16)
        nc.sync.dma_start(wt2_sb[C : 2 * C, :], wt2)
        tcw8_sb = singles.tile([8, NCH, D], f16)
        nc.sync.dma_start(tcw8_sb, aps["tcw8"])
        pw_sb = singles.tile([P, S], f16)
        nc.sync.dma_start(pw_sb, pw)

        # ---- PE priming: absorb each const-DMA wait on a dedicated PE op so
        # later matmuls never carry more than one sync wait.  Only the consts
        # needed by the throw broadcast prime here; the plane/conv weights
        # prime later so they don't stall the startup-critical PE chain.
        prime_h = psT.tile([P, P], f16, tag="pst")
        nc.tensor.transpose(prime_h, identh_sb, identh_sb)
        dumps = singles.tile([P, 4], f32)
        nc.scalar.copy(dumps[:, 0:1], prime_h[:, 0:1])
        nc.tensor.ldweights(selw_sb[:, 0:P])
        nc.tensor.ldweights(wt01_sb[:, 0:P])
        nc.tensor.ldweights(wt2_sb[C : 2 * C, 0:P], tile_position=(64, 0))
        nc.tensor.ldweights(tcw8_sb[:, 0, 0:P])
        nc.tensor.ldweights(pw_sb[:, 0:P])

        # ---- Phase A (per batch): norms/thresh/broadcast + xTA build ----
        st = {}

        def phase_a(b):
            xin_sb = xins[b]
            # norms (L1 over channels), two-level sum (bit-matches reference)
            r8 = small.tile([P, NT, 8], f32, tag="r8", name=f"r8_{b}")
            nc.vector.tensor_reduce(
                r8,
                xin_sb.rearrange("p j (a b) -> p j a b", b=8),
                axis=Ax.X,
                op=Alu.add,
                apply_absolute_value=True,
            )
            normc = small.tile([P, NT], f32, tag="normc", name=f"normc{b}")
            nc.vector.tensor_reduce(normc, r8, axis=Ax.X, op=Alu.add)
            # thresh = (norms + EPS) * 0.7 (exact fp32 rounding order of ref)
            thc = small.tile([P, NT], f32, tag="thc", name=f"thc{b}")
            nc.vector.tensor_scalar(
                thc, normc, float(EPS), float(1.0 - ETA), op0=Alu.add, op1=Alu.mult
            )
            # thresh broadcast over partitions: short DRAM roundtrip for the
            # s-contiguous row, then a K=1 ones-matmul fans it out on the PE
            # (the 1MB partition_broadcast DMA costs ~30us; this costs ~5us)
            # thresh broadcast over partitions: exact 3-way f16 split
            # (hi + mid/4096 + lo/4096 == thc bit-exact; the scaling keeps
            # the f16 components normal), short DRAM roundtrip for the
            # s-contiguous rows, then a K=3 ones-matmul fans out on the PE.
            # (partition_broadcast DMA of the 1MB plane costs ~30us serial.)
            th3 = small.tile([P, NT, 8], f16, tag="th3", name=f"th3_{b}")
            nc.vector.memset(th3, 0)
            r1 = small.tile([P, NT], f32, tag="r1", name=f"r1_{b}")
            r2n = small.tile([P, NT], f32, tag="r2n", name=f"r2n_{b}")
            nc.vector.tensor_copy(th3[:, :, 0], thc)
            nc.vector.tensor_tensor(r1, thc, th3[:, :, 0], op=Alu.subtract)
            nc.vector.tensor_scalar(th3[:, :, 1], r1, 4096.0, None, op0=Alu.mult)
            nc.vector.scalar_tensor_tensor(
                r2n, th3[:, :, 1], float(2.0**-12), r1,
                op0=Alu.mult, op1=Alu.subtract,
            )
            nc.vector.tensor_scalar(th3[:, :, 2], r2n, -4096.0, None, op0=Alu.mult)
            # on-chip permutation: transpose [P, (j k)] -> [(j k), P] puts
            # tile j's three components at partitions 8j..8j+2 (k=3..7 zero);
            # the selw matmul (3 nonzero rows per output tile) fans them out
            th8T_ps = psT.tile([P, P], f16, tag="pst", name=f"th8Tps{b}")
            nc.tensor.transpose(
                th8T_ps, th3.rearrange("p j k -> p (j k)"), identh_sb
            )
            th8T = rowpool.tile([P, P], f16, tag="throwrow", name=f"th8T{b}")
            nc.scalar.copy(th8T, th8T_ps)
            throw = rowpool.tile([P, S], f32, tag="throw", name=f"throw{b}")
            for g in range(NCH):
                thps = psW.tile([P, CW], f32, tag="work", name=f"thps{b}_{g}")
                for m in range(4):
                    j = 4 * g + m
                    nc.tensor.matmul(
                        thps[:, m * P : (m + 1) * P],
                        lhsT=selw_sb[:, j * P : (j + 1) * P],
                        rhs=th8T,
                        start=(m == 0),
                        stop=(m == 3),
                    )
                nc.scalar.copy(throw[:, g * CW : (g + 1) * CW], thps)

            # xTA: fp16 x, channel-major, two stacked tap views.
            # 4 transposes share one PSUM bank, evacuated by 2 wide ACT copies.
            xin16 = xpool.tile([P, NT, C], f16, tag="xin16", name=f"xin16_{b}")
            nc.vector.tensor_copy(xin16, xin_sb)
            nc.tensor.ldweights(xin16[:, 0, :])  # absorb DVE wait for PE
            xTA = xtp.tile([P, S + 1], f16, tag="xta", name=f"xTA{b}")
            for g in range(4):
                xtg = psW.tile([C, 4, P], f16, tag="work", name=f"xtg{b}_{g}")
                for m in range(4):
                    nc.tensor.transpose(
                        xtg[:, m, :], xin16[:, 4 * g + m, :], identh_sb
                    )
                c0 = g * 4 * P
                nc.scalar.copy(
                    xTA[0:C, 1 + c0 : 1 + c0 + 4 * P],
                    xtg.rearrange("p m q -> p (m q)"),
                )
                nc.scalar.copy(
                    xTA[C : 2 * C, c0 : c0 + 4 * P],
                    xtg.rearrange("p m q -> p (m q)"),
                )
            nc.scalar.copy(xTA[0:C, 0:1], xTA[0:C, S : S + 1])
            nc.scalar.copy(xTA[C : 2 * C, S : S + 1], xTA[C : 2 * C, 0:1])
            st[b] = (normc, throw, xTA)

        pe16_sb = singles.tile([P, NT, D], f16)

        # ---- streamed plane + pack + decode + tau + conv ----
        def plane_steps(b, jlist):
            """Emit plane tiles (compare + causal + bit-pack matmuls) for the
            given j's of batch b, plus deferred chunk-close decode work."""
            normc, throw, _ = st[b]
            pks, rcs, taucs = state[b]["pks"], state[b]["rc"], state[b]["tauc"]
            for j in jlist:
                c0 = j * P
                w = S - c0
                gj = gjp.tile([P, S], f16, tag="gj", name=f"gj{b}_{j}")
                # cond[s, t] = thresh[t] > norms[s]  (exact fp32 compare)
                nc.vector.tensor_scalar(
                    gj[:, 0:w], throw[:, c0:S], normc[:, j : j + 1], None,
                    op0=Alu.is_gt,
                )
                # causal mask inside the diagonal 128 columns
                nc.vector.tensor_tensor(
                    gj[:, 0:P], gj[:, 0:P], causal_sb, op=Alu.mult
                )
                # bit-pack matmuls: accumulate 16-wide qualifier masks into
                # every open t-chunk this plane tile touches
                for tcn in range(j // 4, NCH):
                    tlo, thi = max(c0, tcn * CW), (tcn + 1) * CW
                    if j == 0:
                        pk = psPK.tile([P, CW], f32, tag="pk",
                                       name=f"pk{b}_{tcn}")
                        pks[tcn] = pk
                    nc.tensor.matmul(
                        pks[tcn][:, tlo - tcn * CW : thi - tcn * CW],
                        lhsT=pw_sb[:, c0 : c0 + P],
                        rhs=gj[:, tlo - c0 : thi - c0],
                        start=(j == 0),
                        stop=(j == 4 * tcn + 3),
                    )
                # deferred chunk-close: decode chunk (j-7)//4 one chunk late
                # so the DVE never stalls on the ACT evacuation
                if (j + 1) % 4 == 0 and j >= 7:
                    chunk_close(b, (j - 7) // 4)

        def chunk_close(b, tcn):
            """Evacuate + decode chunk tcn of batch b: C16 -> F -> rc -> tau."""
            pks, rc, tauc = state[b]["pks"], state[b]["rc"], state[b]["tauc"]
            c1 = decp.tile([P, CW], f32, tag="c1", name=f"c1_{b}_{tcn}")
            nc.scalar.copy(c1, pks[tcn])
            mask = decp.tile([P, CW], f16, tag="mask", name=f"mask{b}_{tcn}")
            nc.vector.tensor_scalar(mask, c1, 0.0, None, op0=Alu.is_gt)
            eu = decp.tile([P, CW], u32, tag="eu", name=f"eu{b}_{tcn}")
            nc.vector.tensor_scalar(
                eu, c1.bitcast(u32), 23, None, op0=Alu.logical_shift_right
            )
            # F = (exponent + 16q - 126) * mask = (s_max within chunk q) + 1
            # (the uint32 exponent feeds the arithmetic op directly -- only
            # bitVec ops forbid the input dtype cast)
            ff = decp.tile([P, CW], f16, tag="ff", name=f"ff{b}_{tcn}")
            nc.vector.scalar_tensor_tensor(
                ff, eu, qb_sb, mask, op0=Alu.add, op1=Alu.mult
            )
            # cross-partition max via PE transpose + free-dim reduce
            pT = psT.tile([P, 4, P], f16, tag="pst", name=f"pT{b}_{tcn}")
            for m in range(4):
                nc.tensor.transpose(
                    pT[:, m, :], ff[:, m * P : (m + 1) * P], identh_sb
                )
            nc.vector.tensor_reduce(
                rc[:, 4 * tcn : 4 * tcn + 4], pT, axis=Ax.X, op=Alu.max
            )
            # tau = (rc > 0) ? (t + 1 - rc) : 0
            sl = slice(4 * tcn, 4 * tcn + 4)
            m01 = state[b]["m01"]
            td = state[b]["td"]
            nc.vector.tensor_scalar(m01[:, sl], rc[:, sl], 0.0, None, op0=Alu.is_gt)
            nc.vector.tensor_tensor(td[:, sl], t1_sb[:, sl], rc[:, sl],
                                    op=Alu.subtract)
            nc.vector.tensor_tensor(tauc[:, sl], td[:, sl], m01[:, sl],
                                    op=Alu.mult)
            # on-chip tau transport: duplicate each of the 4 tile-columns
            # (for the hi/lo weight rows), transpose to [8, P] at base 0 --
            # tau <= 2047 is exact in fp16
            tauc8 = decp.tile([P, 4, 2], f16, tag="tauc8",
                              name=f"tauc8_{b}_{tcn}")
            nc.vector.tensor_copy(tauc8[:, :, 0], tauc[:, sl])
            nc.vector.tensor_copy(tauc8[:, :, 1], tauc[:, sl])
            tauT_ps = psT.tile([8, P], f16, tag="pst", name=f"tauTps{b}_{tcn}")
            nc.tensor.transpose(
                tauT_ps, tauc8.rearrange("p a k -> p (a k)"), identh_sb
            )
            tauT8 = decp.tile([8, P], f16, tag="tauT8", bufs=4,
                              name=f"tauT8_{b}_{tcn}")
            nc.scalar.copy(tauT8, tauT_ps)
            state[b]["tauT8"][tcn] = tauT8

        def conv_chunk(b, tcn):
            """Conv + pe + tau matmuls, evacuation and output DMA for the 4
            token tiles of chunk tcn of batch b."""
            _, _, xTA = st[b]
            tauT8 = state[b]["tauT8"][tcn]
            if b == 0 and tcn == 0:
                # absorb the pe16 const-DMA wait before the first conv matmul
                nc.tensor.ldweights(pe16_sb[:, 0, 0:P])
            osb4 = outp.tile([P, 4, D], f32, tag="osb", name=f"osb{b}_{tcn}")
            for jj in range(4):
                i = 4 * tcn + jj
                ps = psW.tile([P, D], f32, tag="work", name=f"ps{b}_{i}")
                nc.tensor.matmul(
                    ps, lhsT=identh_sb, rhs=pe16_sb[:, i, :], start=True,
                    stop=False,
                )
                nc.tensor.matmul(
                    ps,
                    lhsT=xTA[:, i * P : (i + 1) * P],
                    rhs=wt01_sb,
                    start=False,
                    stop=False,
                )
                nc.tensor.matmul(
                    ps,
                    lhsT=xTA[C : 2 * C, i * P + 1 : (i + 1) * P + 1],
                    rhs=wt2_sb[C : 2 * C, :],
                    start=False,
                    stop=False,
                )
                nc.tensor.matmul(
                    ps,
                    lhsT=tauT8,
                    rhs=tcw8_sb[:, jj, :],
                    start=False,
                    stop=True,
                    tile_position=(0, 0),
                )
                nc.scalar.copy(osb4[:, jj, :], ps)
            # two half-DMAs on both HWDGE rings: halves the per-chunk
            # completion latency (packets interleave across SDMA engines)
            ov = out[b, tcn * CW : (tcn + 1) * CW, :].rearrange(
                "(jj p) d -> p jj d", p=P
            )
            nc.sync.dma_start(ov[:, 0:2, :], osb4[:, 0:2, :])
            nc.scalar.dma_start(ov[:, 2:4, :], osb4[:, 2:4, :])

        state = {}
        for b in range(BLOC):
            rc = small.tile([P, NT], f32, tag="rc", name=f"rc{b}")
            m01 = small.tile([P, NT], f32, tag="m01", name=f"m01_{b}")
            td = small.tile([P, NT], f32, tag="td", name=f"td{b}")
            tauc = small.tile([P, NT], f32, tag="tauc", name=f"tauc{b}")
            state[b] = dict(pks={}, rc=rc, m01=m01, td=td, tauc=tauc,
                            tauT8={})

        # conv chunks slot into the plane j-loop as soon as their tau
        # transport can have landed, keeping the PE dense (HAM stays warm);
        # batch 0's plane starts before batch 1's phase A, and the pe16 load
        # dispatches after the startup-critical sync-ring traffic
        phase_a(0)
        phase_a(1)
        nc.scalar.dma_start(pe16_sb, pe16.rearrange("(i p) d -> p i d", p=P))
        plane_steps(0, range(0, 12))
        conv_chunk(0, 0)
        plane_steps(0, range(12, 16))
        conv_chunk(0, 1)
        chunk_close(0, 3)
        plane_steps(1, range(0, 4))
        conv_chunk(0, 2)
        plane_steps(1, range(4, 8))
        conv_chunk(0, 3)
        plane_steps(1, range(8, 12))
        conv_chunk(1, 0)
        plane_steps(1, range(12, 16))
        conv_chunk(1, 1)
        chunk_close(1, 3)
        conv_chunk(1, 2)
        conv_chunk(1, 3)


def build_bass():
    """Build the per-core Bass module (traced once, then bacc-compiled)."""
    import concourse.tile as tile
    from concourse import bacc, mybir

    f32 = mybir.dt.float32
    f16 = mybir.dt.float16

    nc = bacc.Bacc(
        "TRN2",
        target_bir_lowering=False,
        debug=False,
        enable_asserts=False,
        num_devices=NCORES,
    )
    aps = {}
    aps["xin"] = nc.dram_tensor("xin", (BLOC, S, C), f32, kind="ExternalInput").ap()
    aps["pe16"] = nc.dram_tensor("pe16", (S, D), f16, kind="ExternalInput").ap()
    aps["wt01"] = nc.dram_tensor("wt01", (P, D), f16, kind="ExternalInput").ap()
    aps["wt2"] = nc.dram_tensor("wt2", (C, D), f16, kind="ExternalInput").ap()
    aps["tcw8"] = nc.dram_tensor("tcw8", (8, NCH, D), f16, kind="ExternalInput").ap()
    aps["identh"] = nc.dram_tensor("identh", (P, P), f16, kind="ExternalInput").ap()
    aps["causal"] = nc.dram_tensor("causal", (P, P), f16, kind="ExternalInput").ap()
    aps["t1c"] = nc.dram_tensor("t1c", (P, NT), f32, kind="ExternalInput").ap()
    aps["pw"] = nc.dram_tensor("pw", (P, S), f16, kind="ExternalInput").ap()
    aps["selw"] = nc.dram_tensor("selw", (P, S), f16, kind="ExternalInput").ap()
    aps["qb"] = nc.dram_tensor("qb", (P, 1), f32, kind="ExternalInput").ap()
    aps["out"] = nc.dram_tensor("out", (BLOC, S, D), f32, kind="ExternalOutput").ap()

    with tile.TileContext(nc) as tc:
        _emit(tc, aps)
    nc.compile()
    return nc


def make_consts():
    """Host-side constant tensors (replicated params + index helpers)."""
    # positional embedding, matching the reference formula in fp32
    position = np.arange(S, dtype=np.float32)[:, None]
    div_term = np.exp(
        np.arange(0, D, 2, dtype=np.float32) * np.float32(-math.log(10000.0) / D)
    ).astype(np.float32)
    ang = (position * div_term).astype(np.float32)
    pe = np.zeros((S, D), dtype=np.float32)
    pe[:, 0::2] = np.sin(ang)
    pe[:, 1::2] = np.cos(ang)

    sl = np.arange(P, dtype=np.float32)[:, None]
    jj = np.arange(NT, dtype=np.float32)[None, :]
    # bit-pack matrix: PW[p, 128j + m] = 2^(p%16) if m == 8j + p//16
    pwm = np.zeros((P, S), dtype=np.float16)
    for j in range(NT):
        for p in range(P):
            pwm[p, 128 * j + 8 * j + p // 16] = np.float16(2.0 ** (p % 16))
    # throw-broadcast selector: column block j selects the 3 split components
    # of tile j (rows 8j..8j+2) with weights (1, 2^-12, 2^-12)
    selw = np.zeros((P, S), dtype=np.float16)
    for j in range(NT):
        for k, w in enumerate((1.0, 2.0**-12, 2.0**-12)):
            selw[8 * j + k, 128 * j : 128 * (j + 1)] = np.float16(w)
    consts = {
        "identh": np.eye(P, dtype=np.float16),
        "causal": (np.arange(P)[:, None] < np.arange(P)[None, :]).astype(np.float16),
        "t1c": (jj * P + sl + 1.0).astype(np.float32),
        "pw": pwm,
        "selw": selw,
        "qb": (16.0 * np.arange(P, dtype=np.float32) - 126.0)[:, None],
    }
    return pe, consts


def make_shared_inputs(conv_w, tc_w, tc_b):
    pe, consts = make_consts()
    pe_b = (pe + np.asarray(tc_b, np.float32)[None, :]).astype(np.float32)
    # conv weights, channel-major per tap: wk[c, d] = conv_w[d, c, k]
    wt = np.transpose(np.asarray(conv_w, np.float32), (2, 1, 0))  # (k, c, d)
    wt01 = np.concatenate([wt[0], wt[1]], axis=0).astype(np.float16)  # (128, D)
    wt2 = wt[2].astype(np.float16)  # (64, D)
    # tc_w split into fp16 hi+lo (tau amplifies errors by up to 2048);
    # tcw8[2jj, jj] = w_hi, tcw8[2jj+1, jj] = w_lo selects tile-in-chunk jj
    w = np.asarray(tc_w, np.float32)[:, 0]
    w_hi = w.astype(np.float16)
    w_lo = (w - w_hi.astype(np.float32)).astype(np.float16)
    tcw8 = np.zeros((8, NCH, D), dtype=np.float16)
    for jj in range(NCH):
        tcw8[2 * jj, jj, :] = w_hi
        tcw8[2 * jj + 1, jj, :] = w_lo
    return {
        "pe16": pe_b.astype(np.float16),
        "wt01": np.ascontiguousarray(wt01),
        "wt2": np.ascontiguousarray(wt2),
        "tcw8": tcw8,
        **{k: np.ascontiguousarray(v) for k, v in consts.items()},
    }


_BUILD_CACHE = {}


def _install_ntff_hook():
    """The agent image's antenv lacks axon_hooks; synthesize it from the
    boot module's ctypes implementation so trace=True works under axon."""
    import sys as _sys
    import types

    if "antenv.axon_hooks" in _sys.modules:
        return
    try:
        from trn_agent_boot.trn_boot import _ntff_profile_via_ctypes

        hook = _ntff_profile_via_ctypes("/opt/axon/libaxon_pjrt.so")
        m = types.ModuleType("antenv.axon_hooks")
        m.get_axon_ntff_profile_hook = lambda: hook
        _sys.modules["antenv.axon_hooks"] = m
    except Exception as e:  # degrade to no-trace
        print("[kernel] ntff hook install failed:", e)


def kernel(x, conv_w, tc_w, tc_b):
    x = np.ascontiguousarray(np.asarray(x, dtype=np.float32))
    conv_w = np.asarray(conv_w, dtype=np.float32)
    tc_w = np.asarray(tc_w, dtype=np.float32)
    tc_b = np.asarray(tc_b, dtype=np.float32)
    assert x.shape == (B, S, C), x.shape

    from concourse.bass_utils import run_bass_kernel_spmd

    if "nc" not in _BUILD_CACHE:
        _BUILD_CACHE["nc"] = build_bass()
    nc = _BUILD_CACHE["nc"]

    shared = make_shared_inputs(conv_w, tc_w, tc_b)
    in_maps = []
    for c in range(NCORES):
        m = dict(shared)
        m["xin"] = np.ascontiguousarray(x[c * BLOC : (c + 1) * BLOC])
        in_maps.append(m)

    trace = bool(int(os.environ.get("KERNEL_TRACE", "0")))
    if trace:
        _install_ntff_hook()
    res = run_bass_kernel_spmd(
        nc, in_maps, core_ids=list(range(NCORES)), trace=trace, trace_cores=[0]
    )
    if trace and res.exec_time_ns is not None:
        print(
            f"[kernel] HW exec time: {res.exec_time_ns} ns "
            f"(mean {res.mean_exec_time_ns} ns)"
        )
        kernel.last_exec_time_ns = res.exec_time_ns
        kernel.last_trace = res.instructions_and_trace
    out = np.concatenate([r["out"] for r in res.results], axis=0)
    return out


if __name__ == "__main__":
    build_bass()
    print("build ok")
